# revision 1
# baseline (speedup 1.0000x reference)
"""Trainium2 Bass kernel for the MoE-routing module.

Computation (B=32768, D=1024, H=512, F=100, E=16, K=2):
    h   = relu(x @ W_shared + b_shared)                  [B, H]
    a   = relu(einsum('bh,ehf', h, W1) + b1)             [B, E, F]
    o   = einsum('bef,efo', a, W2) + b2                  [B, E, 1]
    out = mean over the K routed experts of o[b, send_to[idx[b]]]

Strategy: host sorts tokens by head id and shards the sorted batch over the
8 cores (4096 tokens each, perfectly balanced).  A sorted 4096-token window
only routes to a handful of consecutive experts, so each core gets just the
expert slices it needs (EC slots, adaptively >= actual need; EC=16 degrades
to the dense all-expert kernel).  Routing is folded into a host-computed
per-slot mask M[j, b], so the device computes
    out[b] = sum_j o_local[b, j] * M[j, b]
with three matmul stages, features on SBUF partitions throughout:
  M1: hT[h, t]  = relu(W_shared.T @ xT)         lhsT = W_shared tiles
  M2: aT[f', t] = relu(W1sel.T @ hT)            f' = j*F + f  (EC*F wide)
  M3: c[j, t]   = W2sel.T @ aT                  W2sel block-diagonal
  sel: out[t]   = ones.T @ (c * mask)           1-partition result row
All matmuls run as float32r (full-rate fp32 mode, ~1e-4 rel err).
"""

import os

import numpy as np

import concourse.mybir as mybir
from concourse import bacc
from concourse.bass_utils import run_bass_kernel_spmd
from concourse.tile import TileContext

B, D, H, F, E, TOPK = 32768, 1024, 512, 100, 16, 2
N_CORES = 8
BL = B // N_CORES          # tokens per core
CHUNK = 512                # tokens per device-side tile loop
N_CHUNKS = BL // CHUNK
MH = H // 128              # M1 output tiles
KD = D // 128              # M1 contraction tiles
KH = H // 128              # M2 contraction tiles
EC_MIN = 5                 # minimum expert slots per core
CHUNK_SIZES = [512] * 8

# Compute dtype for the matmul stages: "float32", "float32r", or "bfloat16"
COMPUTE_DT = os.environ.get("KERNEL_DT", "float32r")

_FP32 = mybir.dt.float32
_cache = {}


def _np_in_dtype():
    import ml_dtypes

    return ml_dtypes.bfloat16 if COMPUTE_DT == "bfloat16" else np.float32


def _build_nc(ec):
    """Build the SPMD program for EC expert slots per core."""
    CDT = getattr(mybir.dt, COMPUTE_DT)
    SDT = mybir.dt.bfloat16 if COMPUTE_DT == "bfloat16" else mybir.dt.float32
    EF = ec * F                    # local expert-concat width
    KT3 = (EF + 127) // 128        # M2 output tiles / M3 contraction tiles
    EF_PAD = KT3 * 128             # w1sel zero-padded so all tiles are full
    NB = MH + KT3 + 1              # packed bias columns

    nc = bacc.Bacc("TRN2", target_bir_lowering=False, num_devices=N_CORES)

    xT_d = nc.declare_dram_parameter("xT", [D * BL], CDT, isOutput=False)
    mask_d = nc.declare_dram_parameter("mask", [33, BL], _FP32, isOutput=False)
    wsh_d = nc.declare_dram_parameter("wsh", [D, H], CDT, isOutput=False)
    w1c_d = nc.declare_dram_parameter("w1c", [H, EF_PAD], CDT, isOutput=False)
    w2bd_d = nc.declare_dram_parameter("w2bd", [128, KT3 * ec], CDT, isOutput=False)
    bias_d = nc.declare_dram_parameter("biases", [128, NB], _FP32, isOutput=False)
    out_d = nc.declare_dram_parameter("out", [BL], _FP32, isOutput=True)

    relu = mybir.ActivationFunctionType.Relu
    sizes = CHUNK_SIZES
    offs = np.cumsum([0] + sizes).tolist()

    with TileContext(nc) as tc:
        with (
            tc.tile_pool(name="weights", bufs=1) as wpool,
            tc.tile_pool(name="xin", bufs=3) as xpool,
            tc.tile_pool(name="mid", bufs=3) as midpool,
            tc.tile_pool(name="small", bufs=3) as spool,
            tc.tile_pool(name="ps_h", bufs=4, space="PSUM") as ps_h,
            tc.tile_pool(name="ps_a", bufs=2, space="PSUM") as ps_a,
            tc.tile_pool(name="ps_c", bufs=1, space="PSUM") as ps_c,
            tc.tile_pool(name="ps_o", bufs=1, space="PSUM") as ps_o,
        ):
            # ---- input DMAs: explicit priorities pin queue order to
            # program order.  Separate tiles per k-piece — Tile dependency
            # tracking is per-tile, so split DMAs into one tile would
            # serialize as write-after-write.  wsh + chunk-0 x interleave
            # across both HWDGE queues so M1 starts after the first ~512KB.
            _prio = [0]

            def pdma(q, dst, src):
                inst = q.dma_start(dst, src)
                inst.ins.bass_priority = _prio[0]
                _prio[0] += 1
                return inst

            def xview(c):
                sz = sizes[c]
                o = offs[c] * D
                return xT_d[o : o + D * sz].rearrange("(ko p t) -> p ko t", p=128, t=sz)

            wsh_view = wsh_d.rearrange("(o p) h -> p o h", p=128)
            wsh_ks = [wpool.tile([128, H], CDT, name=f"wshk{k}") for k in range(KD)]
            xt0_view = xview(0)
            xt0 = [
                xpool.tile([128, CHUNK], CDT, tag=f"xt{k}", name=f"xt0_{k}")
                for k in range(KD)
            ]
            for k in range(KD):
                qa = nc.sync if k % 2 == 0 else nc.scalar
                qb = nc.scalar if k % 2 == 0 else nc.sync
                pdma(qa, wsh_ks[k][:], wsh_view[:, k])
                pdma(qb, xt0[k][:, : sizes[0]], xt0_view[:, k])

            xts, masks = [[t[:, : sizes[0]] for t in xt0]], []
            w1c_ks = [None] * KH
            for c in range(len(sizes)):
                sz = sizes[c]
                if c > 0:
                    xv = xview(c)
                    xa = xpool.tile([128, KD // 2, CHUNK], CDT, tag="xta", name=f"xta{c}")
                    xb = xpool.tile([128, KD // 2, CHUNK], CDT, tag="xtb", name=f"xtb{c}")
                    pdma(nc.scalar, xa[:, :, :sz], xv[:, : KD // 2])
                    pdma(nc.sync, xb[:, :, :sz], xv[:, KD // 2 :])
                    xts.append([xa[:, k, :sz] for k in range(KD // 2)] + [xb[:, k, :sz] for k in range(KD // 2)])
                mask_sb = spool.tile([33, CHUNK], _FP32, tag="mask")
                pdma(nc.scalar, mask_sb[:, :sz], mask_d[:, offs[c] : offs[c] + sz])
                masks.append(mask_sb[:, :sz])
                if c == 0:
                    w1c_view = w1c_d.rearrange("(o p) f -> p o f", p=128)
                    for k in range(KH):
                        w1c_ks[k] = wpool.tile([128, EF_PAD], CDT, name=f"w1ck{k}")
                        pdma(nc.sync if k % 2 == 0 else nc.scalar, w1c_ks[k][:], w1c_view[:, k])
                    w2bd_sb = wpool.tile([128, KT3 * ec], CDT)
                    pdma(nc.sync, w2bd_sb[:], w2bd_d[:])
                    bias_sb = wpool.tile([128, NB], _FP32)
                    pdma(nc.sync, bias_sb[:], bias_d[:])
                    ones_sb = wpool.tile([ec, 1], CDT)
                    if COMPUTE_DT == "float32r":
                        nc.vector.memset(ones_sb[:].bitcast(mybir.dt.float32), 1.0)
                    else:
                        nc.vector.memset(ones_sb[:], 1.0)

            for c in range(len(sizes)):
                sz = sizes[c]
                t0 = offs[c]
                xt = xts[c]
                mask_sb = masks[c]

                # ---- M1: hT = relu(W_shared.T @ xT + b) ----
                # chunk 0 runs k-outer so matmuls start as soon as the first
                # split DMA pieces land; later chunks are fully prefetched.
                hT = midpool.tile([128, MH, CHUNK], CDT, tag="hT", name=f"hT{c}")[:, :, :sz]
                if c == 0:
                    phs = [ps_h.tile([128, CHUNK], _FP32, tag="ps_h", name=f"ph{m}")[:, :sz] for m in range(MH)]
                    for k in range(KD):
                        for m in range(MH):
                            nc.tensor.matmul(
                                phs[m][:],
                                lhsT=wsh_ks[k][:, m * 128 : (m + 1) * 128],
                                rhs=xt[k][:],
                                start=(k == 0),
                                stop=(k == KD - 1),
                            )
                    for m in range(MH):
                        nc.scalar.activation(
                            hT[:, m, :], phs[m][:], relu, bias=bias_sb[:, m : m + 1]
                        )
                else:
                    for m in range(MH):
                        ph = ps_h.tile([128, CHUNK], _FP32, tag="ps_h", name=f"phx{c}_{m}")[:, :sz]
                        for k in range(KD):
                            nc.tensor.matmul(
                                ph[:],
                                lhsT=wsh_ks[k][:, m * 128 : (m + 1) * 128],
                                rhs=xt[k][:],
                                start=(k == 0),
                                stop=(k == KD - 1),
                            )
                        nc.scalar.activation(
                            hT[:, m, :], ph[:], relu, bias=bias_sb[:, m : m + 1]
                        )

                # ---- M2: aT = relu(W1sel.T @ hT + b1) ----
                aT = midpool.tile([128, KT3, CHUNK], CDT, tag="aT", name=f"aT{c}")[:, :, :sz]
                for m in range(KT3):
                    f0 = m * 128
                    pa = ps_a.tile([128, CHUNK], _FP32, tag="ps_a", name=f"pa{c}_{m}")[:, :sz]
                    for k in range(KH):
                        nc.tensor.matmul(
                            pa[:],
                            lhsT=w1c_ks[k][:, f0 : f0 + 128],
                            rhs=hT[:, k, :],
                            start=(k == 0),
                            stop=(k == KH - 1),
                        )
                    nc.scalar.activation(
                        aT[:, m, :], pa[:], relu,
                        bias=bias_sb[:, MH + m : MH + m + 1],
                    )

                # ---- M3: c = W2sel.T @ aT  (block-diag W2) ----
                pc = ps_c.tile([ec, CHUNK], _FP32, tag="ps_c", name=f"pc{c}")[:, :sz]
                for k in range(KT3):
                    nc.tensor.matmul(
                        pc[:],
                        lhsT=w2bd_sb[:, k * ec : (k + 1) * ec],
                        rhs=aT[:, k, :],
                        start=(k == 0),
                        stop=(k == KT3 - 1),
                    )

                # ---- select: out = ones.T @ (c * mask) + btok ----
                msel = spool.tile([ec, CHUNK], CDT, tag="msel", name=f"msel{c}")[:, :sz]
                nc.vector.tensor_mul(msel[:], pc[:], mask_sb[:ec])
                po = ps_o.tile([1, CHUNK], _FP32, tag="ps_o", name=f"po{c}")[:, :sz]
                nc.tensor.matmul(po[:], lhsT=ones_sb[:], rhs=msel[:], start=True, stop=True)
                ot = spool.tile([1, CHUNK], _FP32, tag="ot", name=f"ot{c}")[:, :sz]
                nc.vector.tensor_add(ot[:], po[:], mask_sb[32:33])
                nc.gpsimd.dma_start(out_d[t0 : t0 + sz].rearrange("(o t) -> o t", o=1), ot[:])

    nc.compile()
    return nc


def get_nc(ec):
    key = (COMPUTE_DT, ec)
    if key not in _cache:
        _cache[key] = _build_nc(ec)
    return _cache[key]


def prepare(inputs):
    """Host-side routing/sorting/sharding. Returns (ec, in_maps, perm)."""
    np_dt = _np_in_dtype()
    x = np.asarray(inputs["x"], dtype=np.float32)
    idx = np.asarray(inputs["idx"]).astype(np.int64).reshape(B)
    W_shared = np.asarray(inputs["W_shared"], dtype=np.float32)
    b_shared = np.asarray(inputs["b_shared"], dtype=np.float32).reshape(H)
    W1 = np.asarray(inputs["W1"], dtype=np.float32)
    b1 = np.asarray(inputs["b1"], dtype=np.float32).reshape(E, F)
    W2 = np.asarray(inputs["W2"], dtype=np.float32).reshape(E, F)
    b2 = np.asarray(inputs["b2"], dtype=np.float32).reshape(E)
    send_to = np.asarray(inputs["send_to"]).astype(np.int64)

    perm = np.argsort(idx, kind="stable")
    idx_s = idx[perm]
    routes_s = send_to[idx_s]                      # [B, K] sorted routes
    x_s = x[perm]                                  # [B, D]

    # per-core expert lists
    expert_lists = []
    for c in range(N_CORES):
        sl = slice(c * BL, (c + 1) * BL)
        expert_lists.append(np.unique(routes_s[sl]))
    ec = max(EC_MIN, max(len(el) for el in expert_lists))
    ec = min(ec, E)

    wsh = np.ascontiguousarray(W_shared).astype(np_dt)
    EF = ec * F
    KT3 = (EF + 127) // 128
    EF_PAD = KT3 * 128
    NB = MH + KT3 + 1

    in_maps = []
    for c in range(N_CORES):
        sl = slice(c * BL, (c + 1) * BL)
        el = expert_lists[c]
        # local slot tables (pad slots use sentinel -1: zero weights, no mask)
        slots = np.full(ec, -1, dtype=np.int64)
        slots[: len(el)] = el

        # mask[j, b] = (1/K) * count of slots[j] among routes of token b
        r = routes_s[sl]                            # [BL, K]
        mask = np.zeros((33, BL), dtype=np.float32)
        for k in range(r.shape[1]):
            hit = slots[:, None] == r[None, :, k]   # [ec, BL]
            mask[:ec] += hit.astype(np.float32) / r.shape[1]
        mask[32] = b2[r].mean(axis=1)               # routed-b2 mean per token

        w1sel = np.zeros((H, EF_PAD), dtype=np.float32)
        b1sel = np.zeros(EF_PAD, dtype=np.float32)
        w2full = np.zeros((EF_PAD, ec), dtype=np.float32)
        for j, e in enumerate(slots):
            if e < 0:
                continue
            w1sel[:, j * F : (j + 1) * F] = W1[e]
            b1sel[j * F : (j + 1) * F] = b1[e]
            w2full[j * F : (j + 1) * F, j] = W2[e]
        w2bd = np.ascontiguousarray(
            w2full.reshape(KT3, 128, ec).transpose(1, 0, 2).reshape(128, KT3 * ec)
        ).astype(np_dt)

        biases = np.zeros((128, NB), dtype=np.float32)
        biases[:, :MH] = b_shared.reshape(MH, 128).T
        biases[:, MH : MH + KT3] = b1sel.reshape(KT3, 128).T
        biases[:ec, MH + KT3] = b2[np.maximum(slots, 0)] * (slots >= 0)

        xc = x_s[sl]
        parts, o = [], 0
        for szc in CHUNK_SIZES:
            parts.append(xc[o : o + szc].T.ravel())
            o += szc
        xT = np.ascontiguousarray(np.concatenate(parts)).astype(np_dt)

        in_maps.append(
            {
                "xT": xT,
                "mask": mask,
                "wsh": wsh,
                "w1c": w1sel.astype(np_dt),
                "w2bd": w2bd,
                "biases": biases,
            }
        )
    return ec, in_maps, perm


def kernel(**inputs) -> np.ndarray:
    ec, in_maps, perm = prepare(inputs)
    nc = get_nc(ec)
    res = run_bass_kernel_spmd(nc, in_maps, list(range(N_CORES)))
    out_sorted = np.concatenate([res.results[c]["out"] for c in range(N_CORES)])
    out = np.empty(B, dtype=np.float32)
    out[perm] = out_sorted
    return out.reshape(B, 1)



# revision 8
# speedup vs baseline: 1.1270x; 1.1270x over previous
"""Trainium2 Bass kernel for the MoE-routing module.

Computation (B=32768, D=1024, H=512, F=100, E=16, K=2):
    h   = relu(x @ W_shared + b_shared)                  [B, H]
    a   = relu(einsum('bh,ehf', h, W1) + b1)             [B, E, F]
    o   = einsum('bef,efo', a, W2) + b2                  [B, E, 1]
    out = mean over the K routed experts of o[b, send_to[idx[b]]]

Strategy (v2): host sorts tokens by head id.  A 512-token run of sorted
tokens routes to exactly 2 experts when it sits inside one head block
("pair" chunk, 49/64 for uniform heads) and to 3 experts when it spans a
head boundary ("general" chunk, <=15/64).  Chunks are redistributed
across the 8 cores so every core runs the same SPMD chunk-type sequence
SLOT_TYPES (6 pair slots + 2 general slots); per-slot expert weights are
data, so cores differ only in their DRAM contents.

Per chunk, features stay on SBUF partitions:
  M1: hT[h, t]  = relu(W_shared.T @ xT)        8x4 matmuls  (shared)
  M2: aT[f, t]  = relu(W1[e].T @ hT)           4 matmuls per expert tile
  pair path (2 expert tiles):
      out[t]    = w2pair.T @ aT  (+0.5*(b2a+b2b))   2 matmuls, no mask
      (0.5 routing weight folded into w2pair)
  general path (3 expert tiles):
      c[j, t]   = W2blk.T @ aT                 3 matmuls (block diag)
      out[t]    = ones.T @ (c * mask) + b2row  1 matmul + 2 DVE ops
Pair chunks: 42 matmuls; general chunks: 48 (vs 53 for the dense-EC
baseline).  All x is prefetched to SBUF up front (bf16 halves traffic),
so the tensor engine never waits on DMA after the first chunk.
Compute dtype bfloat16 (rel err ~5e-3, tolerance 2e-2).
"""

import os

import numpy as np

import concourse.mybir as mybir
from concourse import bacc
from concourse.bass_utils import run_bass_kernel_spmd
from concourse.tile import TileContext

B, D, H, F, E, TOPK = 32768, 1024, 512, 100, 16, 2
N_CORES = 8
BL = B // N_CORES          # tokens per core
CHUNK = 512                # tokens per device-side tile loop
N_CHUNKS = BL // CHUNK
MH = H // 128              # M1 output tiles
KD = D // 128              # M1 contraction tiles
KH = H // 128              # M2 contraction tiles

SLOT_TYPES = ("P", "G", "G", "P", "P", "P", "P", "P")
NTP, NTG = 2, 3            # expert tiles per pair / general slot

# Compute dtype for the matmul stages: "float32", "float32r", or "bfloat16"
COMPUTE_DT = os.environ.get("KERNEL_DT", "bfloat16")

_FP32 = mybir.dt.float32
_cache = {}

EC_MIN = 5                 # legacy fallback: minimum expert slots per core
CHUNK_SIZES = [512] * 8    # legacy fallback chunk sizes


def _np_in_dtype():
    import ml_dtypes

    return ml_dtypes.bfloat16 if COMPUTE_DT == "bfloat16" else np.float32


def _slot_layout(slot_types):
    """Shared compile-time layout: per slot, per expert tile ->
    (w1 col offset, w2 col offset, bias col).  Returns (layout, NT, W2W,
    NBIAS, pob) where pob is the first pair-output-bias column."""
    layout = []
    w1o, w2o, bcol = 0, 0, MH
    for t in slot_types:
        ntile = NTP if t == "P" else NTG
        w2w = 1 if t == "P" else NTG
        tiles = []
        for _ in range(ntile):
            tiles.append((w1o, w2o, bcol))
            w1o += F
            w2o += w2w
            bcol += 1
        layout.append((t, tiles))
    pob = bcol
    nbias = bcol + len(slot_types)
    return layout, w1o // F, w2o, nbias, pob


def _build_nc_v2():
    """SPMD program for the SLOT_TYPES chunk sequence."""
    CDT = getattr(mybir.dt, COMPUTE_DT)
    layout, NT, W2W, NBIAS, POB = _slot_layout(SLOT_TYPES)
    W1W = NT * F
    NG = sum(1 for t in SLOT_TYPES if t == "G")

    nc = bacc.Bacc("TRN2", target_bir_lowering=False, num_devices=N_CORES)

    xT_d = nc.declare_dram_parameter("xT", [D * BL], CDT, isOutput=False)
    wsh_d = nc.declare_dram_parameter("wsh", [D, H], CDT, isOutput=False)
    w1t_d = nc.declare_dram_parameter("w1t", [H, W1W], CDT, isOutput=False)
    w2t_d = nc.declare_dram_parameter("w2t", [F, W2W], CDT, isOutput=False)
    bias_d = nc.declare_dram_parameter("biases", [128, NBIAS], _FP32, isOutput=False)
    mask_d = nc.declare_dram_parameter("maskg", [NG * 33, CHUNK], _FP32, isOutput=False)
    out_d = nc.declare_dram_parameter("out", [BL], _FP32, isOutput=True)

    relu = mybir.ActivationFunctionType.Relu
    ident = mybir.ActivationFunctionType.Identity

    with TileContext(nc) as tc:
        with (
            tc.tile_pool(name="weights", bufs=1) as wpool,
            tc.tile_pool(name="xin", bufs=1) as xpool,
            tc.tile_pool(name="mid", bufs=3) as midpool,
            tc.tile_pool(name="small", bufs=3) as spool,
            tc.tile_pool(name="ps_h", bufs=4, space="PSUM") as ps_h,
            tc.tile_pool(name="ps_a", bufs=2, space="PSUM") as ps_a,
            tc.tile_pool(name="ps_c", bufs=1, space="PSUM") as ps_c,
            tc.tile_pool(name="ps_o", bufs=1, space="PSUM") as ps_o,
        ):
            # ---- input DMAs: priorities pin queue order to program order.
            # wsh + chunk-0 x interleave across both HWDGE rings so M1
            # starts after the first ~256KB; everything else rides Sync so
            # Scalar is free for activations from chunk 0 on.
            _prio = [0]

            def pdma(q, dst, src):
                inst = q.dma_start(dst, src)
                inst.ins.bass_priority = _prio[0]
                _prio[0] += 1
                return inst

            def xview(c):
                o = c * CHUNK * D
                return xT_d[o : o + D * CHUNK].rearrange(
                    "(ko p t) -> p ko t", p=128, t=CHUNK
                )

            wsh_view = wsh_d.rearrange("(o p) h -> p o h", p=128)
            wsh_ks = [wpool.tile([128, H], CDT, name=f"wshk{k}") for k in range(KD)]
            xt0_view = xview(0)
            xt0 = [
                xpool.tile([128, CHUNK], CDT, tag=f"x0_{k}", name=f"xt0_{k}")
                for k in range(KD)
            ]
            for k in range(KD):
                qa = nc.sync if k % 2 == 0 else nc.scalar
                qb = nc.scalar if k % 2 == 0 else nc.sync
                pdma(qa, wsh_ks[k][:], wsh_view[:, k])
                pdma(qb, xt0[k][:], xt0_view[:, k])

            # chunk-1 x next (needed ~10us after chunk 0's), then W1 table,
            # then the small tables, then the remaining chunks.
            xts = [xt0]
            xt1 = xpool.tile([128, KD, CHUNK], CDT, tag="x1", name="xt1")
            pdma(nc.sync, xt1[:], xview(1))
            xts.append([xt1[:, k, :] for k in range(KD)])

            w1t_view = w1t_d.rearrange("(o p) f -> p o f", p=128)
            w1t_ks = []
            for k in range(KH):
                t = wpool.tile([128, W1W], CDT, name=f"w1tk{k}")
                pdma(nc.sync, t[:], w1t_view[:, k])
                w1t_ks.append(t)
            w2t_sb = wpool.tile([F, W2W], CDT)
            pdma(nc.sync, w2t_sb[:], w2t_d[:])
            bias_sb = wpool.tile([128, NBIAS], _FP32)
            pdma(nc.sync, bias_sb[:], bias_d[:])
            # per-G-slot mask tiles: rows 0..NTG-1 = routing mask, row 32 =
            # routed-b2 mean (offsets 0/32 keep DVE operands quad-aligned)
            mask_sbs = []
            for g in range(NG):
                msk = wpool.tile([33, CHUNK], _FP32, name=f"maskg{g}")
                pdma(nc.sync, msk[:], mask_d[g * 33 : (g + 1) * 33])
                mask_sbs.append(msk)
            ones_sb = wpool.tile([NTG, 1], CDT)
            if COMPUTE_DT == "float32r":
                nc.vector.memset(ones_sb[:].bitcast(mybir.dt.float32), 1.0)
            else:
                nc.vector.memset(ones_sb[:], 1.0)

            for c in range(2, N_CHUNKS):
                xt = xpool.tile([128, KD, CHUNK], CDT, tag=f"x{c}", name=f"xt{c}")
                pdma(nc.sync, xt[:], xview(c))
                xts.append([xt[:, k, :] for k in range(KD)])

            gi = 0  # general-slot counter
            for c, (stype, tiles) in enumerate(layout):
                xt = xts[c]

                # ---- M1: hT = relu(W_shared.T @ xT + b) ----
                # chunk 0 runs k-outer so matmuls start as soon as the first
                # split DMA pieces land; later chunks are fully prefetched.
                hT = midpool.tile([128, MH, CHUNK], CDT, tag="hT", name=f"hT{c}")
                if c == 0:
                    phs = [
                        ps_h.tile([128, CHUNK], _FP32, tag="ps_h", name=f"ph{m}")
                        for m in range(MH)
                    ]
                    for k in range(KD):
                        for m in range(MH):
                            nc.tensor.matmul(
                                phs[m][:],
                                lhsT=wsh_ks[k][:, m * 128 : (m + 1) * 128],
                                rhs=xt[k][:],
                                start=(k == 0),
                                stop=(k == KD - 1),
                            )
                    for m in range(MH):
                        nc.scalar.activation(
                            hT[:, m, :], phs[m][:], relu, bias=bias_sb[:, m : m + 1]
                        )
                else:
                    for m in range(MH):
                        ph = ps_h.tile(
                            [128, CHUNK], _FP32, tag="ps_h", name=f"phx{c}_{m}"
                        )
                        for k in range(KD):
                            nc.tensor.matmul(
                                ph[:],
                                lhsT=wsh_ks[k][:, m * 128 : (m + 1) * 128],
                                rhs=xt[k][:],
                                start=(k == 0),
                                stop=(k == KD - 1),
                            )
                        nc.scalar.activation(
                            hT[:, m, :], ph[:], relu, bias=bias_sb[:, m : m + 1]
                        )

                # ---- M2: per expert tile aT = relu(W1[e].T @ hT + b1) ----
                ntile = len(tiles)
                aT = midpool.tile([F, NTG, CHUNK], CDT, tag="aT", name=f"aT{c}")
                for j, (w1o, w2o, bcol) in enumerate(tiles):
                    pa = ps_a.tile([F, CHUNK], _FP32, tag="ps_a", name=f"pa{c}_{j}")
                    for k in range(KH):
                        nc.tensor.matmul(
                            pa[:],
                            lhsT=w1t_ks[k][:, w1o : w1o + F],
                            rhs=hT[:, k, :],
                            start=(k == 0),
                            stop=(k == KH - 1),
                        )
                    nc.scalar.activation(
                        aT[:, j, :], pa[:], relu, bias=bias_sb[:F, bcol : bcol + 1]
                    )

                t0 = c * CHUNK
                if stype == "P":
                    # ---- pair: out = w2pair.T @ aT + b2mean ----
                    po = ps_o.tile([1, CHUNK], _FP32, tag="ps_o", name=f"po{c}")
                    for j, (w1o, w2o, bcol) in enumerate(tiles):
                        nc.tensor.matmul(
                            po[:],
                            lhsT=w2t_sb[:, w2o : w2o + 1],
                            rhs=aT[:, j, :],
                            start=(j == 0),
                            stop=(j == ntile - 1),
                        )
                    ot = spool.tile([1, CHUNK], _FP32, tag="ot", name=f"ot{c}")
                    nc.scalar.activation(
                        ot[:], po[:], ident, bias=bias_sb[0:1, POB + c : POB + c + 1]
                    )
                else:
                    # ---- general: c = W2blk.T @ aT; masked partition sum ----
                    pc = ps_c.tile([NTG, CHUNK], _FP32, tag="ps_c", name=f"pc{c}")
                    for j, (w1o, w2o, bcol) in enumerate(tiles):
                        nc.tensor.matmul(
                            pc[:],
                            lhsT=w2t_sb[:, w2o : w2o + NTG],
                            rhs=aT[:, j, :],
                            start=(j == 0),
                            stop=(j == ntile - 1),
                        )
                    msel = spool.tile([NTG, CHUNK], CDT, tag="msel", name=f"msel{c}")
                    nc.vector.tensor_mul(msel[:], pc[:], mask_sbs[gi][:NTG])
                    po = ps_o.tile([1, CHUNK], _FP32, tag="ps_o", name=f"pog{c}")
                    nc.tensor.matmul(
                        po[:], lhsT=ones_sb[:], rhs=msel[:], start=True, stop=True
                    )
                    ot = spool.tile([1, CHUNK], _FP32, tag="ot", name=f"ot{c}")
                    nc.vector.tensor_add(ot[:], po[:], mask_sbs[gi][32:33])
                    gi += 1
                nc.gpsimd.dma_start(
                    out_d[t0 : t0 + CHUNK].rearrange("(o t) -> o t", o=1), ot[:]
                )

    nc.compile()
    return nc


def prepare_v2(inputs):
    """Host-side sort/chunk-classify/shard.  Returns (in_maps, unperm) or
    None if the data does not fit the SLOT_TYPES structure."""
    np_dt = _np_in_dtype()
    x = np.asarray(inputs["x"], dtype=np.float32)
    idx = np.asarray(inputs["idx"]).astype(np.int64).reshape(B)
    W_shared = np.asarray(inputs["W_shared"], dtype=np.float32)
    b_shared = np.asarray(inputs["b_shared"], dtype=np.float32).reshape(H)
    W1 = np.asarray(inputs["W1"], dtype=np.float32)
    b1 = np.asarray(inputs["b1"], dtype=np.float32).reshape(E, F)
    W2 = np.asarray(inputs["W2"], dtype=np.float32).reshape(E, F)
    b2 = np.asarray(inputs["b2"], dtype=np.float32).reshape(E)
    send_to = np.asarray(inputs["send_to"]).astype(np.int64)

    perm = np.argsort(idx, kind="stable")
    idx_s = idx[perm]
    routes_s = send_to[idx_s]                      # [B, K] sorted routes
    x_s = x[perm]

    nch = B // CHUNK
    chunk_experts = []
    for cid in range(nch):
        r = routes_s[cid * CHUNK : (cid + 1) * CHUNK]
        chunk_experts.append(np.unique(r))
    pair_pool = [cid for cid in range(nch) if len(chunk_experts[cid]) <= NTP]
    gen_pool = [cid for cid in range(nch) if len(chunk_experts[cid]) == NTG]
    if len(pair_pool) + len(gen_pool) != nch:
        return None                                # some chunk has >3 experts
    n_gslots = sum(1 for t in SLOT_TYPES if t == "G") * N_CORES
    n_pslots = nch - n_gslots
    if len(gen_pool) > n_gslots or len(pair_pool) < n_pslots:
        return None

    layout, NT, W2W, NBIAS, POB = _slot_layout(SLOT_TYPES)
    W1W = NT * F
    NG = sum(1 for t in SLOT_TYPES if t == "G")

    wsh = np.ascontiguousarray(W_shared).astype(np_dt)
    bsh_cols = b_shared.reshape(MH, 128).T

    in_maps, order = [], []
    gp, pp = 0, 0
    for core in range(N_CORES):
        w1t = np.zeros((H, W1W), dtype=np.float32)
        w2t = np.zeros((F, W2W), dtype=np.float32)
        biases = np.zeros((128, NBIAS), dtype=np.float32)
        biases[:, :MH] = bsh_cols
        maskg = np.zeros((NG * 33, CHUNK), dtype=np.float32)
        xc = np.empty((N_CHUNKS, D, CHUNK), dtype=np.float32)
        gi = 0
        for s, (stype, tiles) in enumerate(layout):
            if stype == "G" and gp < len(gen_pool):
                cid = gen_pool[gp]
                gp += 1
            else:
                cid = pair_pool[pp]
                pp += 1
            order.append(cid)
            sl = slice(cid * CHUNK, (cid + 1) * CHUNK)
            els = chunk_experts[cid]
            xc[s] = x_s[sl].T
            r = routes_s[sl]                       # [CHUNK, K]
            if stype == "P":
                # a <=2-expert chunk is single-head: every token routes to
                # the same expert pair, each with weight 1/K.  Fold that
                # weight into w2 and the routed-b2 mean into the out bias.
                if len(els) != NTP or not (r == r[0]).all():
                    return None
                es = list(els)
                for j, (w1o, w2o, bcol) in enumerate(tiles):
                    e = es[j]
                    w1t[:, w1o : w1o + F] = W1[e]
                    biases[:F, bcol] = b1[e]
                    cnt = float((r[0] == e).sum()) / TOPK
                    w2t[:, w2o] = W2[e] * cnt
                biases[0, POB + s] = float(b2[r[0]].sum()) / TOPK
            else:
                es = list(els) + [els[0]] * (NTG - len(els))
                for j, (w1o, w2o, bcol) in enumerate(tiles):
                    e = es[j]
                    w1t[:, w1o : w1o + F] = W1[e]
                    biases[:F, bcol] = b1[e]
                    w2t[:, w2o + j] = W2[e]
                hit = np.zeros((NTG, CHUNK), dtype=np.float32)
                for k in range(r.shape[1]):
                    for j in range(len(els)):
                        hit[j] += (r[:, k] == es[j]).astype(np.float32)
                if len(els) < NTG:                 # dedupe padded tiles
                    hit[len(els):] = 0.0
                maskg[gi * 33 : gi * 33 + NTG] = hit / TOPK
                maskg[gi * 33 + 32] = b2[r].mean(axis=1)
                gi += 1
        in_maps.append(
            {
                "xT": np.ascontiguousarray(xc).astype(np_dt).ravel(),
                "wsh": wsh,
                "w1t": w1t.astype(np_dt),
                "w2t": w2t.astype(np_dt),
                "biases": biases,
                "maskg": maskg,
            }
        )
    # unperm: output concat order -> original token positions
    sorted_pos = np.concatenate(
        [np.arange(cid * CHUNK, (cid + 1) * CHUNK) for cid in order]
    )
    unperm = perm[sorted_pos]
    return in_maps, unperm


def get_nc_v2():
    key = (COMPUTE_DT, "v2", SLOT_TYPES)
    if key not in _cache:
        _cache[key] = _build_nc_v2()
    return _cache[key]


# ---------------------------------------------------------------------------
# legacy dense-EC fallback (used only if the data breaks the v2 structure)
# ---------------------------------------------------------------------------


def _build_nc_legacy(ec):
    """Build the SPMD program for EC expert slots per core."""
    CDT = getattr(mybir.dt, COMPUTE_DT)
    EF = ec * F                    # local expert-concat width
    KT3 = (EF + 127) // 128        # M2 output tiles / M3 contraction tiles
    EF_PAD = KT3 * 128             # w1sel zero-padded so all tiles are full
    NB = MH + KT3 + 1              # packed bias columns

    nc = bacc.Bacc("TRN2", target_bir_lowering=False, num_devices=N_CORES)

    xT_d = nc.declare_dram_parameter("xT", [D * BL], CDT, isOutput=False)
    mask_d = nc.declare_dram_parameter("mask", [33, BL], _FP32, isOutput=False)
    wsh_d = nc.declare_dram_parameter("wsh", [D, H], CDT, isOutput=False)
    w1c_d = nc.declare_dram_parameter("w1c", [H, EF_PAD], CDT, isOutput=False)
    w2bd_d = nc.declare_dram_parameter("w2bd", [128, KT3 * ec], CDT, isOutput=False)
    bias_d = nc.declare_dram_parameter("biases", [128, NB], _FP32, isOutput=False)
    out_d = nc.declare_dram_parameter("out", [BL], _FP32, isOutput=True)

    relu = mybir.ActivationFunctionType.Relu
    sizes = CHUNK_SIZES
    offs = np.cumsum([0] + sizes).tolist()

    with TileContext(nc) as tc:
        with (
            tc.tile_pool(name="weights", bufs=1) as wpool,
            tc.tile_pool(name="xin", bufs=3) as xpool,
            tc.tile_pool(name="mid", bufs=3) as midpool,
            tc.tile_pool(name="small", bufs=3) as spool,
            tc.tile_pool(name="ps_h", bufs=4, space="PSUM") as ps_h,
            tc.tile_pool(name="ps_a", bufs=2, space="PSUM") as ps_a,
            tc.tile_pool(name="ps_c", bufs=1, space="PSUM") as ps_c,
            tc.tile_pool(name="ps_o", bufs=1, space="PSUM") as ps_o,
        ):
            _prio = [0]

            def pdma(q, dst, src):
                inst = q.dma_start(dst, src)
                inst.ins.bass_priority = _prio[0]
                _prio[0] += 1
                return inst

            def xview(c):
                sz = sizes[c]
                o = offs[c] * D
                return xT_d[o : o + D * sz].rearrange("(ko p t) -> p ko t", p=128, t=sz)

            wsh_view = wsh_d.rearrange("(o p) h -> p o h", p=128)
            wsh_ks = [wpool.tile([128, H], CDT, name=f"wshk{k}") for k in range(KD)]
            xt0_view = xview(0)
            xt0 = [
                xpool.tile([128, CHUNK], CDT, tag=f"xt{k}", name=f"xt0_{k}")
                for k in range(KD)
            ]
            for k in range(KD):
                qa = nc.sync if k % 2 == 0 else nc.scalar
                qb = nc.scalar if k % 2 == 0 else nc.sync
                pdma(qa, wsh_ks[k][:], wsh_view[:, k])
                pdma(qb, xt0[k][:, : sizes[0]], xt0_view[:, k])

            xts, masks = [[t[:, : sizes[0]] for t in xt0]], []
            w1c_ks = [None] * KH
            for c in range(len(sizes)):
                sz = sizes[c]
                if c > 0:
                    xv = xview(c)
                    xa = xpool.tile([128, KD // 2, CHUNK], CDT, tag="xta", name=f"xta{c}")
                    xb = xpool.tile([128, KD // 2, CHUNK], CDT, tag="xtb", name=f"xtb{c}")
                    pdma(nc.scalar, xa[:, :, :sz], xv[:, : KD // 2])
                    pdma(nc.sync, xb[:, :, :sz], xv[:, KD // 2 :])
                    xts.append([xa[:, k, :sz] for k in range(KD // 2)] + [xb[:, k, :sz] for k in range(KD // 2)])
                mask_sb = spool.tile([33, CHUNK], _FP32, tag="mask")
                pdma(nc.scalar, mask_sb[:, :sz], mask_d[:, offs[c] : offs[c] + sz])
                masks.append(mask_sb[:, :sz])
                if c == 0:
                    w1c_view = w1c_d.rearrange("(o p) f -> p o f", p=128)
                    for k in range(KH):
                        w1c_ks[k] = wpool.tile([128, EF_PAD], CDT, name=f"w1ck{k}")
                        pdma(nc.sync if k % 2 == 0 else nc.scalar, w1c_ks[k][:], w1c_view[:, k])
                    w2bd_sb = wpool.tile([128, KT3 * ec], CDT)
                    pdma(nc.sync, w2bd_sb[:], w2bd_d[:])
                    bias_sb = wpool.tile([128, NB], _FP32)
                    pdma(nc.sync, bias_sb[:], bias_d[:])
                    ones_sb = wpool.tile([ec, 1], CDT)
                    if COMPUTE_DT == "float32r":
                        nc.vector.memset(ones_sb[:].bitcast(mybir.dt.float32), 1.0)
                    else:
                        nc.vector.memset(ones_sb[:], 1.0)

            for c in range(len(sizes)):
                sz = sizes[c]
                t0 = offs[c]
                xt = xts[c]
                mask_sb = masks[c]

                hT = midpool.tile([128, MH, CHUNK], CDT, tag="hT", name=f"hT{c}")[:, :, :sz]
                if c == 0:
                    phs = [ps_h.tile([128, CHUNK], _FP32, tag="ps_h", name=f"ph{m}")[:, :sz] for m in range(MH)]
                    for k in range(KD):
                        for m in range(MH):
                            nc.tensor.matmul(
                                phs[m][:],
                                lhsT=wsh_ks[k][:, m * 128 : (m + 1) * 128],
                                rhs=xt[k][:],
                                start=(k == 0),
                                stop=(k == KD - 1),
                            )
                    for m in range(MH):
                        nc.scalar.activation(
                            hT[:, m, :], phs[m][:], relu, bias=bias_sb[:, m : m + 1]
                        )
                else:
                    for m in range(MH):
                        ph = ps_h.tile([128, CHUNK], _FP32, tag="ps_h", name=f"phx{c}_{m}")[:, :sz]
                        for k in range(KD):
                            nc.tensor.matmul(
                                ph[:],
                                lhsT=wsh_ks[k][:, m * 128 : (m + 1) * 128],
                                rhs=xt[k][:],
                                start=(k == 0),
                                stop=(k == KD - 1),
                            )
                        nc.scalar.activation(
                            hT[:, m, :], ph[:], relu, bias=bias_sb[:, m : m + 1]
                        )

                aT = midpool.tile([128, KT3, CHUNK], CDT, tag="aT", name=f"aT{c}")[:, :, :sz]
                for m in range(KT3):
                    f0 = m * 128
                    pa = ps_a.tile([128, CHUNK], _FP32, tag="ps_a", name=f"pa{c}_{m}")[:, :sz]
                    for k in range(KH):
                        nc.tensor.matmul(
                            pa[:],
                            lhsT=w1c_ks[k][:, f0 : f0 + 128],
                            rhs=hT[:, k, :],
                            start=(k == 0),
                            stop=(k == KH - 1),
                        )
                    nc.scalar.activation(
                        aT[:, m, :], pa[:], relu,
                        bias=bias_sb[:, MH + m : MH + m + 1],
                    )

                pc = ps_c.tile([ec, CHUNK], _FP32, tag="ps_c", name=f"pc{c}")[:, :sz]
                for k in range(KT3):
                    nc.tensor.matmul(
                        pc[:],
                        lhsT=w2bd_sb[:, k * ec : (k + 1) * ec],
                        rhs=aT[:, k, :],
                        start=(k == 0),
                        stop=(k == KT3 - 1),
                    )

                msel = spool.tile([ec, CHUNK], CDT, tag="msel", name=f"msel{c}")[:, :sz]
                nc.vector.tensor_mul(msel[:], pc[:], mask_sb[:ec])
                po = ps_o.tile([1, CHUNK], _FP32, tag="ps_o", name=f"po{c}")[:, :sz]
                nc.tensor.matmul(po[:], lhsT=ones_sb[:], rhs=msel[:], start=True, stop=True)
                ot = spool.tile([1, CHUNK], _FP32, tag="ot", name=f"ot{c}")[:, :sz]
                nc.vector.tensor_add(ot[:], po[:], mask_sb[32:33])
                nc.gpsimd.dma_start(out_d[t0 : t0 + sz].rearrange("(o t) -> o t", o=1), ot[:])

    nc.compile()
    return nc


def prepare_legacy(inputs):
    """Legacy host-side routing/sorting/sharding. Returns (ec, in_maps, unperm)."""
    np_dt = _np_in_dtype()
    x = np.asarray(inputs["x"], dtype=np.float32)
    idx = np.asarray(inputs["idx"]).astype(np.int64).reshape(B)
    W_shared = np.asarray(inputs["W_shared"], dtype=np.float32)
    b_shared = np.asarray(inputs["b_shared"], dtype=np.float32).reshape(H)
    W1 = np.asarray(inputs["W1"], dtype=np.float32)
    b1 = np.asarray(inputs["b1"], dtype=np.float32).reshape(E, F)
    W2 = np.asarray(inputs["W2"], dtype=np.float32).reshape(E, F)
    b2 = np.asarray(inputs["b2"], dtype=np.float32).reshape(E)
    send_to = np.asarray(inputs["send_to"]).astype(np.int64)

    perm = np.argsort(idx, kind="stable")
    idx_s = idx[perm]
    routes_s = send_to[idx_s]
    x_s = x[perm]

    expert_lists = []
    for c in range(N_CORES):
        sl = slice(c * BL, (c + 1) * BL)
        expert_lists.append(np.unique(routes_s[sl]))
    ec = max(EC_MIN, max(len(el) for el in expert_lists))
    ec = min(ec, E)

    wsh = np.ascontiguousarray(W_shared).astype(np_dt)
    EF = ec * F
    KT3 = (EF + 127) // 128
    EF_PAD = KT3 * 128
    NB = MH + KT3 + 1

    in_maps = []
    for c in range(N_CORES):
        sl = slice(c * BL, (c + 1) * BL)
        el = expert_lists[c]
        slots = np.full(ec, -1, dtype=np.int64)
        slots[: len(el)] = el

        r = routes_s[sl]
        mask = np.zeros((33, BL), dtype=np.float32)
        for k in range(r.shape[1]):
            hit = slots[:, None] == r[None, :, k]
            mask[:ec] += hit.astype(np.float32) / r.shape[1]
        mask[32] = b2[r].mean(axis=1)

        w1sel = np.zeros((H, EF_PAD), dtype=np.float32)
        b1sel = np.zeros(EF_PAD, dtype=np.float32)
        w2full = np.zeros((EF_PAD, ec), dtype=np.float32)
        for j, e in enumerate(slots):
            if e < 0:
                continue
            w1sel[:, j * F : (j + 1) * F] = W1[e]
            b1sel[j * F : (j + 1) * F] = b1[e]
            w2full[j * F : (j + 1) * F, j] = W2[e]
        w2bd = np.ascontiguousarray(
            w2full.reshape(KT3, 128, ec).transpose(1, 0, 2).reshape(128, KT3 * ec)
        ).astype(np_dt)

        biases = np.zeros((128, NB), dtype=np.float32)
        biases[:, :MH] = b_shared.reshape(MH, 128).T
        biases[:, MH : MH + KT3] = b1sel.reshape(KT3, 128).T
        biases[:ec, MH + KT3] = b2[np.maximum(slots, 0)] * (slots >= 0)

        xc = x_s[sl]
        parts, o = [], 0
        for szc in CHUNK_SIZES:
            parts.append(xc[o : o + szc].T.ravel())
            o += szc
        xT = np.ascontiguousarray(np.concatenate(parts)).astype(np_dt)

        in_maps.append(
            {
                "xT": xT,
                "mask": mask,
                "wsh": wsh,
                "w1c": w1sel.astype(np_dt),
                "w2bd": w2bd,
                "biases": biases,
            }
        )
    return ec, in_maps, perm


# ---------------------------------------------------------------------------
# public API
# ---------------------------------------------------------------------------


def prepare(inputs):
    """Returns (key, in_maps, unperm): actual[unperm] = concat(core outs)."""
    v2 = prepare_v2(inputs)
    if v2 is not None:
        in_maps, unperm = v2
        return ("v2",), in_maps, unperm
    ec, in_maps, perm = prepare_legacy(inputs)
    return ("legacy", ec), in_maps, perm


def get_nc(key):
    if key[0] == "v2":
        return get_nc_v2()
    ec = key[1]
    ckey = (COMPUTE_DT, "legacy", ec)
    if ckey not in _cache:
        _cache[ckey] = _build_nc_legacy(ec)
    return _cache[ckey]


def kernel(**inputs) -> np.ndarray:
    key, in_maps, unperm = prepare(inputs)
    nc = get_nc(key)
    res = run_bass_kernel_spmd(nc, in_maps, list(range(N_CORES)))
    out_sorted = np.concatenate([res.results[c]["out"] for c in range(N_CORES)])
    out = np.empty(B, dtype=np.float32)
    out[unperm] = out_sorted
    return out.reshape(B, 1)


# revision 10
# speedup vs baseline: 1.2102x; 1.0738x over previous
"""Trainium2 Bass kernel for the MoE-routing module.

Computation (B=32768, D=1024, H=512, F=100, E=16, K=2):
    h   = relu(x @ W_shared + b_shared)                  [B, H]
    a   = relu(einsum('bh,ehf', h, W1) + b1)             [B, E, F]
    o   = einsum('bef,efo', a, W2) + b2                  [B, E, 1]
    out = mean over the K routed experts of o[b, send_to[idx[b]]]

Strategy (v2): host sorts tokens by head id.  A 512-token run of sorted
tokens routes to exactly 2 experts when it sits inside one head block
("pair" chunk, 49/64 for uniform heads) and to 3 experts when it spans a
head boundary ("general" chunk, <=15/64).  Chunks are redistributed
across the 8 cores so every core runs the same SPMD chunk-type sequence
SLOT_TYPES (6 pair slots + 2 general slots); per-slot expert weights are
data, so cores differ only in their DRAM contents.

Per chunk, features stay on SBUF partitions:
  M1: hT[h, t]  = relu(W_shared.T @ xT)        8x4 matmuls  (shared)
  M2: aT[f, t]  = relu(W1[e].T @ hT)           4 matmuls per expert tile
  pair path (2 expert tiles):
      out[t]    = w2pair.T @ aT  (+0.5*(b2a+b2b))   2 matmuls, no mask
      (0.5 routing weight folded into w2pair)
  general path (3 expert tiles):
      c[j, t]   = W2blk.T @ aT                 3 matmuls (block diag)
      out[t]    = ones.T @ (c * mask) + b2row  1 matmul + 2 DVE ops
Pair chunks: 42 matmuls; general chunks: 48 (vs 53 for the dense-EC
baseline).  All x is prefetched to SBUF up front (bf16 halves traffic),
so the tensor engine never waits on DMA after the first chunk.
Compute dtype bfloat16 (rel err ~5e-3, tolerance 2e-2).
"""

import os

import numpy as np

import concourse.mybir as mybir
from concourse import bacc
from concourse.bass_utils import run_bass_kernel_spmd
from concourse.tile import TileContext

B, D, H, F, E, TOPK = 32768, 1024, 512, 100, 16, 2
N_CORES = 8
BL = B // N_CORES          # tokens per core
CHUNK = 512                # tokens per device-side tile loop
N_CHUNKS = BL // CHUNK
MH = H // 128              # M1 output tiles
KD = D // 128              # M1 contraction tiles
KH = H // 128              # M2 contraction tiles

SLOT_TYPES = ("P", "G", "G", "P", "P", "P", "P", "P")
NTP, NTG = 2, 3            # expert tiles per pair / general slot

# Compute dtype for the matmul stages: "float32", "float32r", or "bfloat16"
COMPUTE_DT = os.environ.get("KERNEL_DT", "bfloat16")

_FP32 = mybir.dt.float32
_cache = {}

EC_MIN = 5                 # legacy fallback: minimum expert slots per core
CHUNK_SIZES = [512] * 8    # legacy fallback chunk sizes


def _np_in_dtype():
    import ml_dtypes

    return ml_dtypes.bfloat16 if COMPUTE_DT == "bfloat16" else np.float32


def _slot_layout(slot_types):
    """Shared compile-time layout: per slot, per expert tile ->
    (w1 col offset, w2 col offset, bias col).  Returns (layout, NT, W2W,
    NBIAS, pob) where pob is the first pair-output-bias column."""
    layout = []
    w1o, w2o, bcol = 0, 0, MH
    for t in slot_types:
        ntile = NTP if t == "P" else NTG
        w2w = 1 if t == "P" else NTG
        tiles = []
        for _ in range(ntile):
            tiles.append((w1o, w2o, bcol))
            w1o += F
            w2o += w2w
            bcol += 1
        layout.append((t, tiles))
    pob = bcol
    nbias = bcol + len(slot_types)
    return layout, w1o // F, w2o, nbias, pob


def _build_nc_v2():
    """SPMD program for the SLOT_TYPES chunk sequence."""
    CDT = getattr(mybir.dt, COMPUTE_DT)
    layout, NT, W2W, NBIAS, POB = _slot_layout(SLOT_TYPES)
    W1W = NT * F
    NG = sum(1 for t in SLOT_TYPES if t == "G")

    nc = bacc.Bacc("TRN2", target_bir_lowering=False, num_devices=N_CORES)

    xT_d = nc.declare_dram_parameter("xT", [D * BL], CDT, isOutput=False)
    wsh_d = nc.declare_dram_parameter("wsh", [D, H], CDT, isOutput=False)
    w1t_d = nc.declare_dram_parameter("w1t", [H, W1W], CDT, isOutput=False)
    w2t_d = nc.declare_dram_parameter("w2t", [F, W2W], CDT, isOutput=False)
    bias_d = nc.declare_dram_parameter("biases", [128, NBIAS], _FP32, isOutput=False)
    mask_d = nc.declare_dram_parameter("maskg", [NG * 33, CHUNK], _FP32, isOutput=False)
    out_d = nc.declare_dram_parameter("out", [BL], _FP32, isOutput=True)

    relu = mybir.ActivationFunctionType.Relu
    ident = mybir.ActivationFunctionType.Identity

    with TileContext(nc) as tc:
        with (
            tc.tile_pool(name="weights", bufs=1) as wpool,
            tc.tile_pool(name="xin", bufs=1) as xpool,
            tc.tile_pool(name="mid", bufs=3) as midpool,
            tc.tile_pool(name="small", bufs=3) as spool,
            tc.tile_pool(name="ps_h", bufs=4, space="PSUM") as ps_h,
            tc.tile_pool(name="ps_a", bufs=2, space="PSUM") as ps_a,
            tc.tile_pool(name="ps_c", bufs=1, space="PSUM") as ps_c,
            tc.tile_pool(name="ps_o", bufs=1, space="PSUM") as ps_o,
        ):
            # ---- input DMAs: priorities pin queue order to program order.
            # wsh + chunk-0 x interleave across both HWDGE rings so M1
            # starts after the first ~256KB; everything else rides Sync so
            # Scalar is free for activations from chunk 0 on.
            _prio = [0]

            def pdma(q, dst, src):
                inst = q.dma_start(dst, src)
                inst.ins.bass_priority = _prio[0]
                _prio[0] += 1
                return inst

            def xview(c):
                o = c * CHUNK * D
                return xT_d[o : o + D * CHUNK].rearrange(
                    "(ko p t) -> p ko t", p=128, t=CHUNK
                )

            # startup lacing: chunk-0 pieces interleave across both rings so
            # M1 starts after the first ~256KB and never starves; Scalar's
            # ring opens with the act-table load, so its first piece is the
            # k=1 x (needed 4 matmuls in).  Then bias (chunk-0 ACTs), the
            # split chunk-1 x, the W1 table on both rings, small tables,
            # and the remaining chunks.
            wsh_view = wsh_d.rearrange("(o p) h -> p o h", p=128)
            wsh_ks = [wpool.tile([128, H], CDT, name=f"wshk{k}") for k in range(KD)]
            xt0_view = xview(0)
            xt0 = [
                xpool.tile([128, CHUNK], CDT, tag=f"x0_{k}", name=f"xt0_{k}")
                for k in range(KD)
            ]
            sync_seq = [(wsh_ks[0], 'w', 0), (xt0[0], 'x', 0), (wsh_ks[1], 'w', 1),
                        (xt0[2], 'x', 2), (wsh_ks[3], 'w', 3), (xt0[4], 'x', 4),
                        (wsh_ks[5], 'w', 5), (xt0[6], 'x', 6)]
            scalar_seq = [(xt0[1], 'x', 1), (wsh_ks[2], 'w', 2), (xt0[3], 'x', 3),
                          (wsh_ks[4], 'w', 4), (xt0[5], 'x', 5), (wsh_ks[6], 'w', 6),
                          (xt0[7], 'x', 7), (wsh_ks[7], 'w', 7)]
            for (ta, kinda, ka), (tb, kindb, kb) in zip(sync_seq, scalar_seq):
                pdma(nc.sync, ta[:], wsh_view[:, ka] if kinda == 'w' else xt0_view[:, ka])
                pdma(nc.scalar, tb[:], wsh_view[:, kb] if kindb == 'w' else xt0_view[:, kb])

            bias_sb = wpool.tile([128, NBIAS], _FP32)
            pdma(nc.sync, bias_sb[:], bias_d[:])

            xts = [xt0]
            xt1 = xpool.tile([128, KD, CHUNK], CDT, tag="x1", name="xt1")
            pdma(nc.sync, xt1[:, : KD // 2], xview(1)[:, : KD // 2])
            pdma(nc.sync, xt1[:, KD // 2 :], xview(1)[:, KD // 2 :])
            xts.append([xt1[:, k, :] for k in range(KD)])

            w1t_view = w1t_d.rearrange("(o p) f -> p o f", p=128)
            w1t_ks = [wpool.tile([128, W1W], CDT, name=f"w1tk{k}") for k in range(KH)]
            pdma(nc.scalar, w1t_ks[1][:], w1t_view[:, 1])
            pdma(nc.sync, w1t_ks[0][:], w1t_view[:, 0])
            pdma(nc.scalar, w1t_ks[3][:], w1t_view[:, 3])
            pdma(nc.sync, w1t_ks[2][:], w1t_view[:, 2])
            w2t_sb = wpool.tile([F, W2W], CDT)
            pdma(nc.sync, w2t_sb[:], w2t_d[:])
            # per-G-slot mask tiles: rows 0..NTG-1 = routing mask, row 32 =
            # routed-b2 mean (offsets 0/32 keep DVE operands quad-aligned)
            mask_sbs = []
            for g in range(NG):
                msk = wpool.tile([33, CHUNK], _FP32, name=f"maskg{g}")
                pdma(nc.sync, msk[:], mask_d[g * 33 : (g + 1) * 33])
                mask_sbs.append(msk)
            ones_sb = wpool.tile([NTG, 1], CDT)
            if COMPUTE_DT == "float32r":
                nc.vector.memset(ones_sb[:].bitcast(mybir.dt.float32), 1.0)
            else:
                nc.vector.memset(ones_sb[:], 1.0)

            for c in range(2, N_CHUNKS):
                xt = xpool.tile([128, KD, CHUNK], CDT, tag=f"x{c}", name=f"xt{c}")
                pdma(nc.sync, xt[:], xview(c))
                xts.append([xt[:, k, :] for k in range(KD)])

            # ---- PE schedule: M1 pipelined two chunks ahead ----
            # M1(0) runs k-outer so matmuls start as soon as the first split
            # DMA pieces land; M1(1) follows immediately (its x and the W1
            # table stream in meanwhile), then each iteration c runs
            # M2(c) / M1(c+2) / M3(c), so M2 never waits on the W1-table DMA
            # and M3 never waits on M2's activations.  A general chunk's
            # masked-sum matmul is deferred one iteration to hide the
            # PSUM->DVE->SBUF round trip.
            hTs, aTs = {}, {}

            def emit_m1(c):
                xt = xts[c]
                hT = midpool.tile([128, MH, CHUNK], CDT, tag="hT", name=f"hT{c}")
                hTs[c] = hT
                if c == 0:
                    phs = [
                        ps_h.tile([128, CHUNK], _FP32, tag="ps_h", name=f"ph{m}")
                        for m in range(MH)
                    ]
                    for k in range(KD):
                        for m in range(MH):
                            nc.tensor.matmul(
                                phs[m][:],
                                lhsT=wsh_ks[k][:, m * 128 : (m + 1) * 128],
                                rhs=xt[k][:],
                                start=(k == 0),
                                stop=(k == KD - 1),
                            )
                    for m in range(MH):
                        nc.scalar.activation(
                            hT[:, m, :], phs[m][:], relu, bias=bias_sb[:, m : m + 1]
                        )
                else:
                    for m in range(MH):
                        ph = ps_h.tile(
                            [128, CHUNK], _FP32, tag="ps_h", name=f"phx{c}_{m}"
                        )
                        for k in range(KD):
                            nc.tensor.matmul(
                                ph[:],
                                lhsT=wsh_ks[k][:, m * 128 : (m + 1) * 128],
                                rhs=xt[k][:],
                                start=(k == 0),
                                stop=(k == KD - 1),
                            )
                        nc.scalar.activation(
                            hT[:, m, :], ph[:], relu, bias=bias_sb[:, m : m + 1]
                        )

            emit_m1(0)
            emit_m1(1)

            gi = 0  # general-slot counter
            pending_sel = None
            for c, (stype, tiles) in enumerate(layout):
                if pending_sel is not None:
                    pending_sel()
                    pending_sel = None

                # ---- M2: per expert tile aT = relu(W1[e].T @ hT + b1) ----
                ntile = len(tiles)
                hT = hTs.pop(c)
                aT = midpool.tile([F, NTG, CHUNK], CDT, tag="aT", name=f"aT{c}")
                for j, (w1o, w2o, bcol) in enumerate(tiles):
                    pa = ps_a.tile([F, CHUNK], _FP32, tag="ps_a", name=f"pa{c}_{j}")
                    for k in range(KH):
                        nc.tensor.matmul(
                            pa[:],
                            lhsT=w1t_ks[k][:, w1o : w1o + F],
                            rhs=hT[:, k, :],
                            start=(k == 0),
                            stop=(k == KH - 1),
                        )
                    nc.scalar.activation(
                        aT[:, j, :], pa[:], relu, bias=bias_sb[:F, bcol : bcol + 1]
                    )

                if c + 2 < N_CHUNKS:
                    emit_m1(c + 2)

                t0 = c * CHUNK
                if stype == "P":
                    # ---- pair: out = w2pair.T @ aT + b2mean ----
                    po = ps_o.tile([1, CHUNK], _FP32, tag="ps_o", name=f"po{c}")
                    for j, (w1o, w2o, bcol) in enumerate(tiles):
                        nc.tensor.matmul(
                            po[:],
                            lhsT=w2t_sb[:, w2o : w2o + 1],
                            rhs=aT[:, j, :],
                            start=(j == 0),
                            stop=(j == ntile - 1),
                        )
                    ot = spool.tile([1, CHUNK], _FP32, tag="ot", name=f"ot{c}")
                    nc.scalar.activation(
                        ot[:], po[:], ident, bias=bias_sb[0:1, POB + c : POB + c + 1]
                    )
                    nc.gpsimd.dma_start(
                        out_d[t0 : t0 + CHUNK].rearrange("(o t) -> o t", o=1), ot[:]
                    )
                else:
                    # ---- general: c = W2blk.T @ aT; masked partition sum ----
                    pc = ps_c.tile([NTG, CHUNK], _FP32, tag="ps_c", name=f"pc{c}")
                    for j, (w1o, w2o, bcol) in enumerate(tiles):
                        nc.tensor.matmul(
                            pc[:],
                            lhsT=w2t_sb[:, w2o : w2o + NTG],
                            rhs=aT[:, j, :],
                            start=(j == 0),
                            stop=(j == ntile - 1),
                        )
                    msel = spool.tile([NTG, CHUNK], CDT, tag="msel", name=f"msel{c}")
                    nc.vector.tensor_mul(msel[:], pc[:], mask_sbs[gi][:NTG])
                    g = gi
                    gi += 1

                    def make_sel(cc, gg, msel_t):
                        def emit_sel():
                            po = ps_o.tile(
                                [1, CHUNK], _FP32, tag="ps_o", name=f"pog{cc}"
                            )
                            nc.tensor.matmul(
                                po[:], lhsT=ones_sb[:], rhs=msel_t[:],
                                start=True, stop=True,
                            )
                            ot = spool.tile([1, CHUNK], _FP32, tag="ot", name=f"ot{cc}")
                            nc.vector.tensor_add(ot[:], po[:], mask_sbs[gg][32:33])
                            nc.gpsimd.dma_start(
                                out_d[cc * CHUNK : (cc + 1) * CHUNK].rearrange(
                                    "(o t) -> o t", o=1
                                ),
                                ot[:],
                            )
                        return emit_sel

                    pending_sel = make_sel(c, g, msel)
            if pending_sel is not None:
                pending_sel()

    nc.compile()
    return nc


def prepare_v2(inputs):
    """Host-side sort/chunk-classify/shard.  Returns (in_maps, unperm) or
    None if the data does not fit the SLOT_TYPES structure."""
    np_dt = _np_in_dtype()
    x = np.asarray(inputs["x"], dtype=np.float32)
    idx = np.asarray(inputs["idx"]).astype(np.int64).reshape(B)
    W_shared = np.asarray(inputs["W_shared"], dtype=np.float32)
    b_shared = np.asarray(inputs["b_shared"], dtype=np.float32).reshape(H)
    W1 = np.asarray(inputs["W1"], dtype=np.float32)
    b1 = np.asarray(inputs["b1"], dtype=np.float32).reshape(E, F)
    W2 = np.asarray(inputs["W2"], dtype=np.float32).reshape(E, F)
    b2 = np.asarray(inputs["b2"], dtype=np.float32).reshape(E)
    send_to = np.asarray(inputs["send_to"]).astype(np.int64)

    perm = np.argsort(idx, kind="stable")
    idx_s = idx[perm]
    routes_s = send_to[idx_s]                      # [B, K] sorted routes
    x_s = x[perm]

    nch = B // CHUNK
    chunk_experts = []
    for cid in range(nch):
        r = routes_s[cid * CHUNK : (cid + 1) * CHUNK]
        chunk_experts.append(np.unique(r))
    pair_pool = [cid for cid in range(nch) if len(chunk_experts[cid]) <= NTP]
    gen_pool = [cid for cid in range(nch) if len(chunk_experts[cid]) == NTG]
    if len(pair_pool) + len(gen_pool) != nch:
        return None                                # some chunk has >3 experts
    n_gslots = sum(1 for t in SLOT_TYPES if t == "G") * N_CORES
    n_pslots = nch - n_gslots
    if len(gen_pool) > n_gslots or len(pair_pool) < n_pslots:
        return None

    layout, NT, W2W, NBIAS, POB = _slot_layout(SLOT_TYPES)
    W1W = NT * F
    NG = sum(1 for t in SLOT_TYPES if t == "G")

    wsh = np.ascontiguousarray(W_shared).astype(np_dt)
    bsh_cols = b_shared.reshape(MH, 128).T

    in_maps, order = [], []
    gp, pp = 0, 0
    for core in range(N_CORES):
        w1t = np.zeros((H, W1W), dtype=np.float32)
        w2t = np.zeros((F, W2W), dtype=np.float32)
        biases = np.zeros((128, NBIAS), dtype=np.float32)
        biases[:, :MH] = bsh_cols
        maskg = np.zeros((NG * 33, CHUNK), dtype=np.float32)
        xc = np.empty((N_CHUNKS, D, CHUNK), dtype=np.float32)
        gi = 0
        for s, (stype, tiles) in enumerate(layout):
            if stype == "G" and gp < len(gen_pool):
                cid = gen_pool[gp]
                gp += 1
            else:
                cid = pair_pool[pp]
                pp += 1
            order.append(cid)
            sl = slice(cid * CHUNK, (cid + 1) * CHUNK)
            els = chunk_experts[cid]
            xc[s] = x_s[sl].T
            r = routes_s[sl]                       # [CHUNK, K]
            if stype == "P":
                # a <=2-expert chunk is single-head: every token routes to
                # the same expert pair, each with weight 1/K.  Fold that
                # weight into w2 and the routed-b2 mean into the out bias.
                if len(els) != NTP or not (r == r[0]).all():
                    return None
                es = list(els)
                for j, (w1o, w2o, bcol) in enumerate(tiles):
                    e = es[j]
                    w1t[:, w1o : w1o + F] = W1[e]
                    biases[:F, bcol] = b1[e]
                    cnt = float((r[0] == e).sum()) / TOPK
                    w2t[:, w2o] = W2[e] * cnt
                biases[0, POB + s] = float(b2[r[0]].sum()) / TOPK
            else:
                es = list(els) + [els[0]] * (NTG - len(els))
                for j, (w1o, w2o, bcol) in enumerate(tiles):
                    e = es[j]
                    w1t[:, w1o : w1o + F] = W1[e]
                    biases[:F, bcol] = b1[e]
                    w2t[:, w2o + j] = W2[e]
                hit = np.zeros((NTG, CHUNK), dtype=np.float32)
                for k in range(r.shape[1]):
                    for j in range(len(els)):
                        hit[j] += (r[:, k] == es[j]).astype(np.float32)
                if len(els) < NTG:                 # dedupe padded tiles
                    hit[len(els):] = 0.0
                maskg[gi * 33 : gi * 33 + NTG] = hit / TOPK
                maskg[gi * 33 + 32] = b2[r].mean(axis=1)
                gi += 1
        in_maps.append(
            {
                "xT": np.ascontiguousarray(xc).astype(np_dt).ravel(),
                "wsh": wsh,
                "w1t": w1t.astype(np_dt),
                "w2t": w2t.astype(np_dt),
                "biases": biases,
                "maskg": maskg,
            }
        )
    # unperm: output concat order -> original token positions
    sorted_pos = np.concatenate(
        [np.arange(cid * CHUNK, (cid + 1) * CHUNK) for cid in order]
    )
    unperm = perm[sorted_pos]
    return in_maps, unperm


def get_nc_v2():
    key = (COMPUTE_DT, "v2", SLOT_TYPES)
    if key not in _cache:
        _cache[key] = _build_nc_v2()
    return _cache[key]


# ---------------------------------------------------------------------------
# legacy dense-EC fallback (used only if the data breaks the v2 structure)
# ---------------------------------------------------------------------------


def _build_nc_legacy(ec):
    """Build the SPMD program for EC expert slots per core."""
    CDT = getattr(mybir.dt, COMPUTE_DT)
    EF = ec * F                    # local expert-concat width
    KT3 = (EF + 127) // 128        # M2 output tiles / M3 contraction tiles
    EF_PAD = KT3 * 128             # w1sel zero-padded so all tiles are full
    NB = MH + KT3 + 1              # packed bias columns

    nc = bacc.Bacc("TRN2", target_bir_lowering=False, num_devices=N_CORES)

    xT_d = nc.declare_dram_parameter("xT", [D * BL], CDT, isOutput=False)
    mask_d = nc.declare_dram_parameter("mask", [33, BL], _FP32, isOutput=False)
    wsh_d = nc.declare_dram_parameter("wsh", [D, H], CDT, isOutput=False)
    w1c_d = nc.declare_dram_parameter("w1c", [H, EF_PAD], CDT, isOutput=False)
    w2bd_d = nc.declare_dram_parameter("w2bd", [128, KT3 * ec], CDT, isOutput=False)
    bias_d = nc.declare_dram_parameter("biases", [128, NB], _FP32, isOutput=False)
    out_d = nc.declare_dram_parameter("out", [BL], _FP32, isOutput=True)

    relu = mybir.ActivationFunctionType.Relu
    sizes = CHUNK_SIZES
    offs = np.cumsum([0] + sizes).tolist()

    with TileContext(nc) as tc:
        with (
            tc.tile_pool(name="weights", bufs=1) as wpool,
            tc.tile_pool(name="xin", bufs=3) as xpool,
            tc.tile_pool(name="mid", bufs=3) as midpool,
            tc.tile_pool(name="small", bufs=3) as spool,
            tc.tile_pool(name="ps_h", bufs=4, space="PSUM") as ps_h,
            tc.tile_pool(name="ps_a", bufs=2, space="PSUM") as ps_a,
            tc.tile_pool(name="ps_c", bufs=1, space="PSUM") as ps_c,
            tc.tile_pool(name="ps_o", bufs=1, space="PSUM") as ps_o,
        ):
            _prio = [0]

            def pdma(q, dst, src):
                inst = q.dma_start(dst, src)
                inst.ins.bass_priority = _prio[0]
                _prio[0] += 1
                return inst

            def xview(c):
                sz = sizes[c]
                o = offs[c] * D
                return xT_d[o : o + D * sz].rearrange("(ko p t) -> p ko t", p=128, t=sz)

            wsh_view = wsh_d.rearrange("(o p) h -> p o h", p=128)
            wsh_ks = [wpool.tile([128, H], CDT, name=f"wshk{k}") for k in range(KD)]
            xt0_view = xview(0)
            xt0 = [
                xpool.tile([128, CHUNK], CDT, tag=f"xt{k}", name=f"xt0_{k}")
                for k in range(KD)
            ]
            for k in range(KD):
                qa = nc.sync if k % 2 == 0 else nc.scalar
                qb = nc.scalar if k % 2 == 0 else nc.sync
                pdma(qa, wsh_ks[k][:], wsh_view[:, k])
                pdma(qb, xt0[k][:, : sizes[0]], xt0_view[:, k])

            xts, masks = [[t[:, : sizes[0]] for t in xt0]], []
            w1c_ks = [None] * KH
            for c in range(len(sizes)):
                sz = sizes[c]
                if c > 0:
                    xv = xview(c)
                    xa = xpool.tile([128, KD // 2, CHUNK], CDT, tag="xta", name=f"xta{c}")
                    xb = xpool.tile([128, KD // 2, CHUNK], CDT, tag="xtb", name=f"xtb{c}")
                    pdma(nc.scalar, xa[:, :, :sz], xv[:, : KD // 2])
                    pdma(nc.sync, xb[:, :, :sz], xv[:, KD // 2 :])
                    xts.append([xa[:, k, :sz] for k in range(KD // 2)] + [xb[:, k, :sz] for k in range(KD // 2)])
                mask_sb = spool.tile([33, CHUNK], _FP32, tag="mask")
                pdma(nc.scalar, mask_sb[:, :sz], mask_d[:, offs[c] : offs[c] + sz])
                masks.append(mask_sb[:, :sz])
                if c == 0:
                    w1c_view = w1c_d.rearrange("(o p) f -> p o f", p=128)
                    for k in range(KH):
                        w1c_ks[k] = wpool.tile([128, EF_PAD], CDT, name=f"w1ck{k}")
                        pdma(nc.sync if k % 2 == 0 else nc.scalar, w1c_ks[k][:], w1c_view[:, k])
                    w2bd_sb = wpool.tile([128, KT3 * ec], CDT)
                    pdma(nc.sync, w2bd_sb[:], w2bd_d[:])
                    bias_sb = wpool.tile([128, NB], _FP32)
                    pdma(nc.sync, bias_sb[:], bias_d[:])
                    ones_sb = wpool.tile([ec, 1], CDT)
                    if COMPUTE_DT == "float32r":
                        nc.vector.memset(ones_sb[:].bitcast(mybir.dt.float32), 1.0)
                    else:
                        nc.vector.memset(ones_sb[:], 1.0)

            for c in range(len(sizes)):
                sz = sizes[c]
                t0 = offs[c]
                xt = xts[c]
                mask_sb = masks[c]

                hT = midpool.tile([128, MH, CHUNK], CDT, tag="hT", name=f"hT{c}")[:, :, :sz]
                if c == 0:
                    phs = [ps_h.tile([128, CHUNK], _FP32, tag="ps_h", name=f"ph{m}")[:, :sz] for m in range(MH)]
                    for k in range(KD):
                        for m in range(MH):
                            nc.tensor.matmul(
                                phs[m][:],
                                lhsT=wsh_ks[k][:, m * 128 : (m + 1) * 128],
                                rhs=xt[k][:],
                                start=(k == 0),
                                stop=(k == KD - 1),
                            )
                    for m in range(MH):
                        nc.scalar.activation(
                            hT[:, m, :], phs[m][:], relu, bias=bias_sb[:, m : m + 1]
                        )
                else:
                    for m in range(MH):
                        ph = ps_h.tile([128, CHUNK], _FP32, tag="ps_h", name=f"phx{c}_{m}")[:, :sz]
                        for k in range(KD):
                            nc.tensor.matmul(
                                ph[:],
                                lhsT=wsh_ks[k][:, m * 128 : (m + 1) * 128],
                                rhs=xt[k][:],
                                start=(k == 0),
                                stop=(k == KD - 1),
                            )
                        nc.scalar.activation(
                            hT[:, m, :], ph[:], relu, bias=bias_sb[:, m : m + 1]
                        )

                aT = midpool.tile([128, KT3, CHUNK], CDT, tag="aT", name=f"aT{c}")[:, :, :sz]
                for m in range(KT3):
                    f0 = m * 128
                    pa = ps_a.tile([128, CHUNK], _FP32, tag="ps_a", name=f"pa{c}_{m}")[:, :sz]
                    for k in range(KH):
                        nc.tensor.matmul(
                            pa[:],
                            lhsT=w1c_ks[k][:, f0 : f0 + 128],
                            rhs=hT[:, k, :],
                            start=(k == 0),
                            stop=(k == KH - 1),
                        )
                    nc.scalar.activation(
                        aT[:, m, :], pa[:], relu,
                        bias=bias_sb[:, MH + m : MH + m + 1],
                    )

                pc = ps_c.tile([ec, CHUNK], _FP32, tag="ps_c", name=f"pc{c}")[:, :sz]
                for k in range(KT3):
                    nc.tensor.matmul(
                        pc[:],
                        lhsT=w2bd_sb[:, k * ec : (k + 1) * ec],
                        rhs=aT[:, k, :],
                        start=(k == 0),
                        stop=(k == KT3 - 1),
                    )

                msel = spool.tile([ec, CHUNK], CDT, tag="msel", name=f"msel{c}")[:, :sz]
                nc.vector.tensor_mul(msel[:], pc[:], mask_sb[:ec])
                po = ps_o.tile([1, CHUNK], _FP32, tag="ps_o", name=f"po{c}")[:, :sz]
                nc.tensor.matmul(po[:], lhsT=ones_sb[:], rhs=msel[:], start=True, stop=True)
                ot = spool.tile([1, CHUNK], _FP32, tag="ot", name=f"ot{c}")[:, :sz]
                nc.vector.tensor_add(ot[:], po[:], mask_sb[32:33])
                nc.gpsimd.dma_start(out_d[t0 : t0 + sz].rearrange("(o t) -> o t", o=1), ot[:])

    nc.compile()
    return nc


def prepare_legacy(inputs):
    """Legacy host-side routing/sorting/sharding. Returns (ec, in_maps, unperm)."""
    np_dt = _np_in_dtype()
    x = np.asarray(inputs["x"], dtype=np.float32)
    idx = np.asarray(inputs["idx"]).astype(np.int64).reshape(B)
    W_shared = np.asarray(inputs["W_shared"], dtype=np.float32)
    b_shared = np.asarray(inputs["b_shared"], dtype=np.float32).reshape(H)
    W1 = np.asarray(inputs["W1"], dtype=np.float32)
    b1 = np.asarray(inputs["b1"], dtype=np.float32).reshape(E, F)
    W2 = np.asarray(inputs["W2"], dtype=np.float32).reshape(E, F)
    b2 = np.asarray(inputs["b2"], dtype=np.float32).reshape(E)
    send_to = np.asarray(inputs["send_to"]).astype(np.int64)

    perm = np.argsort(idx, kind="stable")
    idx_s = idx[perm]
    routes_s = send_to[idx_s]
    x_s = x[perm]

    expert_lists = []
    for c in range(N_CORES):
        sl = slice(c * BL, (c + 1) * BL)
        expert_lists.append(np.unique(routes_s[sl]))
    ec = max(EC_MIN, max(len(el) for el in expert_lists))
    ec = min(ec, E)

    wsh = np.ascontiguousarray(W_shared).astype(np_dt)
    EF = ec * F
    KT3 = (EF + 127) // 128
    EF_PAD = KT3 * 128
    NB = MH + KT3 + 1

    in_maps = []
    for c in range(N_CORES):
        sl = slice(c * BL, (c + 1) * BL)
        el = expert_lists[c]
        slots = np.full(ec, -1, dtype=np.int64)
        slots[: len(el)] = el

        r = routes_s[sl]
        mask = np.zeros((33, BL), dtype=np.float32)
        for k in range(r.shape[1]):
            hit = slots[:, None] == r[None, :, k]
            mask[:ec] += hit.astype(np.float32) / r.shape[1]
        mask[32] = b2[r].mean(axis=1)

        w1sel = np.zeros((H, EF_PAD), dtype=np.float32)
        b1sel = np.zeros(EF_PAD, dtype=np.float32)
        w2full = np.zeros((EF_PAD, ec), dtype=np.float32)
        for j, e in enumerate(slots):
            if e < 0:
                continue
            w1sel[:, j * F : (j + 1) * F] = W1[e]
            b1sel[j * F : (j + 1) * F] = b1[e]
            w2full[j * F : (j + 1) * F, j] = W2[e]
        w2bd = np.ascontiguousarray(
            w2full.reshape(KT3, 128, ec).transpose(1, 0, 2).reshape(128, KT3 * ec)
        ).astype(np_dt)

        biases = np.zeros((128, NB), dtype=np.float32)
        biases[:, :MH] = b_shared.reshape(MH, 128).T
        biases[:, MH : MH + KT3] = b1sel.reshape(KT3, 128).T
        biases[:ec, MH + KT3] = b2[np.maximum(slots, 0)] * (slots >= 0)

        xc = x_s[sl]
        parts, o = [], 0
        for szc in CHUNK_SIZES:
            parts.append(xc[o : o + szc].T.ravel())
            o += szc
        xT = np.ascontiguousarray(np.concatenate(parts)).astype(np_dt)

        in_maps.append(
            {
                "xT": xT,
                "mask": mask,
                "wsh": wsh,
                "w1c": w1sel.astype(np_dt),
                "w2bd": w2bd,
                "biases": biases,
            }
        )
    return ec, in_maps, perm


# ---------------------------------------------------------------------------
# public API
# ---------------------------------------------------------------------------


def prepare(inputs):
    """Returns (key, in_maps, unperm): actual[unperm] = concat(core outs)."""
    v2 = prepare_v2(inputs)
    if v2 is not None:
        in_maps, unperm = v2
        return ("v2",), in_maps, unperm
    ec, in_maps, perm = prepare_legacy(inputs)
    return ("legacy", ec), in_maps, perm


def get_nc(key):
    if key[0] == "v2":
        return get_nc_v2()
    ec = key[1]
    ckey = (COMPUTE_DT, "legacy", ec)
    if ckey not in _cache:
        _cache[ckey] = _build_nc_legacy(ec)
    return _cache[ckey]


def kernel(**inputs) -> np.ndarray:
    key, in_maps, unperm = prepare(inputs)
    nc = get_nc(key)
    res = run_bass_kernel_spmd(nc, in_maps, list(range(N_CORES)))
    out_sorted = np.concatenate([res.results[c]["out"] for c in range(N_CORES)])
    out = np.empty(B, dtype=np.float32)
    out[unperm] = out_sorted
    return out.reshape(B, 1)


# revision 16
# speedup vs baseline: 1.2192x; 1.0074x over previous
"""Trainium2 Bass kernel for the MoE-routing module.

Computation (B=32768, D=1024, H=512, F=100, E=16, K=2):
    h   = relu(x @ W_shared + b_shared)                  [B, H]
    a   = relu(einsum('bh,ehf', h, W1) + b1)             [B, E, F]
    o   = einsum('bef,efo', a, W2) + b2                  [B, E, 1]
    out = mean over the K routed experts of o[b, send_to[idx[b]]]

Strategy (v2): host sorts tokens by head id.  A 512-token run of sorted
tokens routes to exactly 2 experts when it sits inside one head block
("pair" chunk, 49/64 for uniform heads) and to 3 experts when it spans a
head boundary ("general" chunk, <=15/64).  Chunks are redistributed
across the 8 cores so every core runs the same SPMD chunk-type sequence
SLOT_TYPES (6 pair slots + 2 general slots); per-slot expert weights are
data, so cores differ only in their DRAM contents.

Per chunk, features stay on SBUF partitions:
  M1: hT[h, t]  = relu(W_shared.T @ xT)        8x4 matmuls  (shared)
  M2: aT[f, t]  = relu(W1[e].T @ hT)           4 matmuls per expert tile
  pair path (2 expert tiles):
      out[t]    = w2pair.T @ aT  (+0.5*(b2a+b2b))   2 matmuls, no mask
      (0.5 routing weight folded into w2pair)
  general path (3 expert tiles):
      c[j, t]   = W2blk.T @ aT                 3 matmuls (block diag)
      out[t]    = ones.T @ (c * mask) + b2row  1 matmul + 2 DVE ops
Pair chunks: 42 matmuls; general chunks: 48 (vs 53 for the dense-EC
baseline).  All x is prefetched to SBUF up front (bf16 halves traffic),
so the tensor engine never waits on DMA after the first chunk.
Compute dtype bfloat16 (rel err ~5e-3, tolerance 2e-2).
"""

import os

import numpy as np

import concourse.mybir as mybir
from concourse import bacc
from concourse.bass_utils import run_bass_kernel_spmd
from concourse.tile import TileContext

B, D, H, F, E, TOPK = 32768, 1024, 512, 100, 16, 2
N_CORES = 8
BL = B // N_CORES          # tokens per core
CHUNK = 512                # tokens per device-side tile loop
N_CHUNKS = BL // CHUNK
MH = H // 128              # M1 output tiles
KD = D // 128              # M1 contraction tiles
KH = H // 128              # M2 contraction tiles

SLOT_TYPES = ("P", "G", "G", "P", "P", "P", "P", "P")
NTP, NTG = 2, 3            # expert tiles per pair / general slot

# Compute dtype for the matmul stages: "float32", "float32r", or "bfloat16"
COMPUTE_DT = os.environ.get("KERNEL_DT", "bfloat16")

_FP32 = mybir.dt.float32
_cache = {}

EC_MIN = 5                 # legacy fallback: minimum expert slots per core
CHUNK_SIZES = [512] * 8    # legacy fallback chunk sizes


def _np_in_dtype():
    import ml_dtypes

    return ml_dtypes.bfloat16 if COMPUTE_DT == "bfloat16" else np.float32


def _slot_layout(slot_types):
    """Shared compile-time layout: per slot, per expert tile ->
    (w1 col offset, w2 col offset, bias col).  Returns (layout, NT, W2W,
    NBIAS, pob) where pob is the first pair-output-bias column."""
    layout = []
    w1o, w2o, bcol = 0, 0, MH
    for t in slot_types:
        ntile = NTP if t == "P" else NTG
        w2w = 1 if t == "P" else NTG
        tiles = []
        for _ in range(ntile):
            tiles.append((w1o, w2o, bcol))
            w1o += F
            w2o += w2w
            bcol += 1
        layout.append((t, tiles))
    pob = bcol
    nbias = bcol + len(slot_types)
    return layout, w1o // F, w2o, nbias, pob


def _build_nc_v2():
    """SPMD program for the SLOT_TYPES chunk sequence."""
    CDT = getattr(mybir.dt, COMPUTE_DT)
    layout, NT, W2W, NBIAS, POB = _slot_layout(SLOT_TYPES)
    W1W = NT * F
    NG = sum(1 for t in SLOT_TYPES if t == "G")

    nc = bacc.Bacc("TRN2", target_bir_lowering=False, num_devices=N_CORES)

    xT_d = nc.declare_dram_parameter("xT", [D * BL], CDT, isOutput=False)
    wsh_d = nc.declare_dram_parameter("wsh", [D, H], CDT, isOutput=False)
    w1t_d = nc.declare_dram_parameter("w1t", [H, W1W], CDT, isOutput=False)
    w2t_d = nc.declare_dram_parameter("w2t", [F, W2W], CDT, isOutput=False)
    bias_d = nc.declare_dram_parameter("biases", [128, NBIAS], _FP32, isOutput=False)
    mask_d = nc.declare_dram_parameter("maskg", [NG * 33, CHUNK], _FP32, isOutput=False)
    out_d = nc.declare_dram_parameter("out", [BL], _FP32, isOutput=True)

    relu = mybir.ActivationFunctionType.Relu
    ident = mybir.ActivationFunctionType.Identity

    with TileContext(nc) as tc:
        with (
            tc.tile_pool(name="weights", bufs=1) as wpool,
            tc.tile_pool(name="xin", bufs=1) as xpool,
            tc.tile_pool(name="mid", bufs=3) as midpool,
            tc.tile_pool(name="small", bufs=3) as spool,
            tc.tile_pool(name="ps_h", bufs=4, space="PSUM") as ps_h,
            tc.tile_pool(name="ps_a", bufs=2, space="PSUM") as ps_a,
            tc.tile_pool(name="ps_c", bufs=1, space="PSUM") as ps_c,
            tc.tile_pool(name="ps_o", bufs=1, space="PSUM") as ps_o,
        ):
            # ---- input DMAs: priorities pin queue order to program order.
            # wsh + chunk-0 x interleave across both HWDGE rings so M1
            # starts after the first ~256KB; everything else rides Sync so
            # Scalar is free for activations from chunk 0 on.
            _prio = [0]

            def pdma(q, dst, src):
                inst = q.dma_start(dst, src)
                inst.ins.bass_priority = _prio[0]
                _prio[0] += 1
                return inst

            def xview(c):
                o = c * CHUNK * D
                return xT_d[o : o + D * CHUNK].rearrange(
                    "(ko p t) -> p ko t", p=128, t=CHUNK
                )

            # startup lacing: chunk-0 pieces interleave across both rings so
            # M1 starts after the first ~256KB; the early DMA subsystem has
            # a ~3-4us warm-up before data flows regardless of piece size,
            # which the PE warm-up matmuls below bridge.  Then bias (chunk-0
            # ACTs), the chunk-1 x and the W1 table split across both rings,
            # small tables, and the remaining chunks.
            wsh_view = wsh_d.rearrange("(o p) h -> p o h", p=128)
            wsh_ks = [wpool.tile([128, H], CDT, name=f"wshk{k}") for k in range(KD)]
            xt0_view = xview(0)
            xt0 = [
                xpool.tile([128, CHUNK], CDT, tag=f"x0_{k}", name=f"xt0_{k}")
                for k in range(KD)
            ]
            for k in range(KD):
                qa = nc.sync if k % 2 == 0 else nc.scalar
                qb = nc.scalar if k % 2 == 0 else nc.sync
                pdma(qa, wsh_ks[k][:], wsh_view[:, k])
                pdma(qb, xt0[k][:], xt0_view[:, k])

            bias_sb = wpool.tile([128, NBIAS], _FP32)
            pdma(nc.sync, bias_sb[:], bias_d[:])

            xts = [xt0]
            xt1 = xpool.tile([128, KD, CHUNK], CDT, tag="x1", name="xt1")
            pdma(nc.sync, xt1[:, : KD // 2], xview(1)[:, : KD // 2])
            pdma(nc.scalar, xt1[:, KD // 2 :], xview(1)[:, KD // 2 :])
            xts.append([xt1[:, k, :] for k in range(KD)])

            w1t_view = w1t_d.rearrange("(o p) f -> p o f", p=128)
            w1t_ks = [wpool.tile([128, W1W], CDT, name=f"w1tk{k}") for k in range(KH)]
            pdma(nc.sync, w1t_ks[0][:], w1t_view[:, 0])
            pdma(nc.scalar, w1t_ks[1][:], w1t_view[:, 1])
            pdma(nc.sync, w1t_ks[2][:], w1t_view[:, 2])
            pdma(nc.scalar, w1t_ks[3][:], w1t_view[:, 3])
            w2t_sb = wpool.tile([F, W2W], CDT)
            pdma(nc.sync, w2t_sb[:], w2t_d[:])
            # per-G-slot mask tiles: rows 0..NTG-1 = routing mask, row 32 =
            # routed-b2 mean (offsets 0/32 keep DVE operands quad-aligned)
            mask_sbs = []
            for g in range(NG):
                msk = wpool.tile([33, CHUNK], _FP32, name=f"maskg{g}")
                pdma(nc.sync, msk[:], mask_d[g * 33 : (g + 1) * 33])
                mask_sbs.append(msk)
            ones_sb = wpool.tile([NTG, 1], CDT)
            if COMPUTE_DT == "float32r":
                nc.vector.memset(ones_sb[:].bitcast(mybir.dt.float32), 1.0)
            else:
                nc.vector.memset(ones_sb[:], 1.0)

            for c in range(2, N_CHUNKS):
                xt = xpool.tile([128, KD, CHUNK], CDT, tag=f"x{c}", name=f"xt{c}")
                pdma(nc.sync, xt[:], xview(c))
                xts.append([xt[:, k, :] for k in range(KD)])

            # ---- PE schedule: M1 pipelined two chunks ahead ----
            # M1(0) runs k-outer so matmuls start as soon as the first split
            # DMA pieces land; M1(1) follows immediately (its x and the W1
            # table stream in meanwhile), then each iteration c runs
            # M2(c) / M1(c+2) / M3(c), so M2 never waits on the W1-table DMA
            # and M3 never waits on M2's activations.  A general chunk's
            # masked-sum matmul is deferred one iteration to hide the
            # PSUM->DVE->SBUF round trip.
            hTs, aTs = {}, {}

            # warm-up: keep the PE continuously busy from the start of the
            # body so the clock ramps to full speed before real data lands
            # (the tensor engine needs ~3us of back-to-back work to leave
            # the mid p-state, and the first x piece takes ~4us to arrive).
            scratch = wpool.tile([4, CHUNK], CDT, name="warm_scratch")
            if COMPUTE_DT == "float32r":
                nc.vector.memset(scratch[:].bitcast(mybir.dt.float32), 1.0)
            else:
                nc.vector.memset(scratch[:], 1.0)
            for i in range(12):
                pw = ps_h.tile([1, CHUNK], _FP32, tag="ps_h", name=f"warm{i}")
                nc.tensor.matmul(
                    pw[:], lhsT=scratch[:, :1], rhs=scratch[:], start=True, stop=True
                )

            def emit_m1(c):
                xt = xts[c]
                hT = midpool.tile([128, MH, CHUNK], CDT, tag="hT", name=f"hT{c}")
                hTs[c] = hT
                if c == 0:
                    phs = [
                        ps_h.tile([128, CHUNK], _FP32, tag="ps_h", name=f"ph{m}")
                        for m in range(MH)
                    ]
                    for k in range(KD):
                        for m in range(MH):
                            nc.tensor.matmul(
                                phs[m][:],
                                lhsT=wsh_ks[k][:, m * 128 : (m + 1) * 128],
                                rhs=xt[k][:],
                                start=(k == 0),
                                stop=(k == KD - 1),
                            )
                    for m in range(MH):
                        nc.scalar.activation(
                            hT[:, m, :], phs[m][:], relu, bias=bias_sb[:, m : m + 1]
                        )
                else:
                    for m in range(MH):
                        ph = ps_h.tile(
                            [128, CHUNK], _FP32, tag="ps_h", name=f"phx{c}_{m}"
                        )
                        for k in range(KD):
                            nc.tensor.matmul(
                                ph[:],
                                lhsT=wsh_ks[k][:, m * 128 : (m + 1) * 128],
                                rhs=xt[k][:],
                                start=(k == 0),
                                stop=(k == KD - 1),
                            )
                        nc.scalar.activation(
                            hT[:, m, :], ph[:], relu, bias=bias_sb[:, m : m + 1]
                        )

            emit_m1(0)
            emit_m1(1)

            gi = 0  # general-slot counter
            pending_sel = None
            for c, (stype, tiles) in enumerate(layout):
                if pending_sel is not None:
                    pending_sel()
                    pending_sel = None

                # ---- M2: per expert tile aT = relu(W1[e].T @ hT + b1) ----
                ntile = len(tiles)
                hT = hTs.pop(c)
                aT = midpool.tile([F, NTG, CHUNK], CDT, tag="aT", name=f"aT{c}")
                for j, (w1o, w2o, bcol) in enumerate(tiles):
                    pa = ps_a.tile([F, CHUNK], _FP32, tag="ps_a", name=f"pa{c}_{j}")
                    for k in range(KH):
                        nc.tensor.matmul(
                            pa[:],
                            lhsT=w1t_ks[k][:, w1o : w1o + F],
                            rhs=hT[:, k, :],
                            start=(k == 0),
                            stop=(k == KH - 1),
                        )
                    nc.scalar.activation(
                        aT[:, j, :], pa[:], relu, bias=bias_sb[:F, bcol : bcol + 1]
                    )

                if c + 2 < N_CHUNKS:
                    emit_m1(c + 2)

                t0 = c * CHUNK
                if stype == "P":
                    # ---- pair: out = w2pair.T @ aT + b2mean ----
                    po = ps_o.tile([1, CHUNK], _FP32, tag="ps_o", name=f"po{c}")
                    for j, (w1o, w2o, bcol) in enumerate(tiles):
                        nc.tensor.matmul(
                            po[:],
                            lhsT=w2t_sb[:, w2o : w2o + 1],
                            rhs=aT[:, j, :],
                            start=(j == 0),
                            stop=(j == ntile - 1),
                        )
                    ot = spool.tile([1, CHUNK], _FP32, tag="ot", name=f"ot{c}")
                    nc.scalar.activation(
                        ot[:], po[:], ident, bias=bias_sb[0:1, POB + c : POB + c + 1]
                    )
                    nc.gpsimd.dma_start(
                        out_d[t0 : t0 + CHUNK].rearrange("(o t) -> o t", o=1), ot[:]
                    )
                else:
                    # ---- general: c = W2blk.T @ aT; masked partition sum ----
                    pc = ps_c.tile([NTG, CHUNK], _FP32, tag="ps_c", name=f"pc{c}")
                    for j, (w1o, w2o, bcol) in enumerate(tiles):
                        nc.tensor.matmul(
                            pc[:],
                            lhsT=w2t_sb[:, w2o : w2o + NTG],
                            rhs=aT[:, j, :],
                            start=(j == 0),
                            stop=(j == ntile - 1),
                        )
                    msel = spool.tile([NTG, CHUNK], CDT, tag="msel", name=f"msel{c}")
                    nc.vector.tensor_mul(msel[:], pc[:], mask_sbs[gi][:NTG])
                    g = gi
                    gi += 1

                    def make_sel(cc, gg, msel_t):
                        def emit_sel():
                            po = ps_o.tile(
                                [1, CHUNK], _FP32, tag="ps_o", name=f"pog{cc}"
                            )
                            nc.tensor.matmul(
                                po[:], lhsT=ones_sb[:], rhs=msel_t[:],
                                start=True, stop=True,
                            )
                            ot = spool.tile([1, CHUNK], _FP32, tag="ot", name=f"ot{cc}")
                            nc.vector.tensor_add(ot[:], po[:], mask_sbs[gg][32:33])
                            nc.gpsimd.dma_start(
                                out_d[cc * CHUNK : (cc + 1) * CHUNK].rearrange(
                                    "(o t) -> o t", o=1
                                ),
                                ot[:],
                            )
                        return emit_sel

                    pending_sel = make_sel(c, g, msel)
            if pending_sel is not None:
                pending_sel()

    nc.compile()
    return nc


def prepare_v2(inputs):
    """Host-side sort/chunk-classify/shard.  Returns (in_maps, unperm) or
    None if the data does not fit the SLOT_TYPES structure."""
    np_dt = _np_in_dtype()
    x = np.asarray(inputs["x"], dtype=np.float32)
    idx = np.asarray(inputs["idx"]).astype(np.int64).reshape(B)
    W_shared = np.asarray(inputs["W_shared"], dtype=np.float32)
    b_shared = np.asarray(inputs["b_shared"], dtype=np.float32).reshape(H)
    W1 = np.asarray(inputs["W1"], dtype=np.float32)
    b1 = np.asarray(inputs["b1"], dtype=np.float32).reshape(E, F)
    W2 = np.asarray(inputs["W2"], dtype=np.float32).reshape(E, F)
    b2 = np.asarray(inputs["b2"], dtype=np.float32).reshape(E)
    send_to = np.asarray(inputs["send_to"]).astype(np.int64)

    perm = np.argsort(idx, kind="stable")
    idx_s = idx[perm]
    routes_s = send_to[idx_s]                      # [B, K] sorted routes
    x_s = x[perm]

    nch = B // CHUNK
    chunk_experts = []
    for cid in range(nch):
        r = routes_s[cid * CHUNK : (cid + 1) * CHUNK]
        chunk_experts.append(np.unique(r))
    pair_pool = [cid for cid in range(nch) if len(chunk_experts[cid]) <= NTP]
    gen_pool = [cid for cid in range(nch) if len(chunk_experts[cid]) == NTG]
    if len(pair_pool) + len(gen_pool) != nch:
        return None                                # some chunk has >3 experts
    n_gslots = sum(1 for t in SLOT_TYPES if t == "G") * N_CORES
    n_pslots = nch - n_gslots
    if len(gen_pool) > n_gslots or len(pair_pool) < n_pslots:
        return None

    layout, NT, W2W, NBIAS, POB = _slot_layout(SLOT_TYPES)
    W1W = NT * F
    NG = sum(1 for t in SLOT_TYPES if t == "G")

    wsh = np.ascontiguousarray(W_shared).astype(np_dt)
    bsh_cols = b_shared.reshape(MH, 128).T

    in_maps, order = [], []
    gp, pp = 0, 0
    for core in range(N_CORES):
        w1t = np.zeros((H, W1W), dtype=np.float32)
        w2t = np.zeros((F, W2W), dtype=np.float32)
        biases = np.zeros((128, NBIAS), dtype=np.float32)
        biases[:, :MH] = bsh_cols
        maskg = np.zeros((NG * 33, CHUNK), dtype=np.float32)
        xc = np.empty((N_CHUNKS, D, CHUNK), dtype=np.float32)
        gi = 0
        for s, (stype, tiles) in enumerate(layout):
            if stype == "G" and gp < len(gen_pool):
                cid = gen_pool[gp]
                gp += 1
            else:
                cid = pair_pool[pp]
                pp += 1
            order.append(cid)
            sl = slice(cid * CHUNK, (cid + 1) * CHUNK)
            els = chunk_experts[cid]
            xc[s] = x_s[sl].T
            r = routes_s[sl]                       # [CHUNK, K]
            if stype == "P":
                # a <=2-expert chunk is single-head: every token routes to
                # the same expert pair, each with weight 1/K.  Fold that
                # weight into w2 and the routed-b2 mean into the out bias.
                if len(els) != NTP or not (r == r[0]).all():
                    return None
                es = list(els)
                for j, (w1o, w2o, bcol) in enumerate(tiles):
                    e = es[j]
                    w1t[:, w1o : w1o + F] = W1[e]
                    biases[:F, bcol] = b1[e]
                    cnt = float((r[0] == e).sum()) / TOPK
                    w2t[:, w2o] = W2[e] * cnt
                biases[0, POB + s] = float(b2[r[0]].sum()) / TOPK
            else:
                es = list(els) + [els[0]] * (NTG - len(els))
                for j, (w1o, w2o, bcol) in enumerate(tiles):
                    e = es[j]
                    w1t[:, w1o : w1o + F] = W1[e]
                    biases[:F, bcol] = b1[e]
                    w2t[:, w2o + j] = W2[e]
                hit = np.zeros((NTG, CHUNK), dtype=np.float32)
                for k in range(r.shape[1]):
                    for j in range(len(els)):
                        hit[j] += (r[:, k] == es[j]).astype(np.float32)
                if len(els) < NTG:                 # dedupe padded tiles
                    hit[len(els):] = 0.0
                maskg[gi * 33 : gi * 33 + NTG] = hit / TOPK
                maskg[gi * 33 + 32] = b2[r].mean(axis=1)
                gi += 1
        in_maps.append(
            {
                "xT": np.ascontiguousarray(xc).astype(np_dt).ravel(),
                "wsh": wsh,
                "w1t": w1t.astype(np_dt),
                "w2t": w2t.astype(np_dt),
                "biases": biases,
                "maskg": maskg,
            }
        )
    # unperm: output concat order -> original token positions
    sorted_pos = np.concatenate(
        [np.arange(cid * CHUNK, (cid + 1) * CHUNK) for cid in order]
    )
    unperm = perm[sorted_pos]
    return in_maps, unperm


def get_nc_v2():
    key = (COMPUTE_DT, "v2", SLOT_TYPES)
    if key not in _cache:
        _cache[key] = _build_nc_v2()
    return _cache[key]


# ---------------------------------------------------------------------------
# legacy dense-EC fallback (used only if the data breaks the v2 structure)
# ---------------------------------------------------------------------------


def _build_nc_legacy(ec):
    """Build the SPMD program for EC expert slots per core."""
    CDT = getattr(mybir.dt, COMPUTE_DT)
    EF = ec * F                    # local expert-concat width
    KT3 = (EF + 127) // 128        # M2 output tiles / M3 contraction tiles
    EF_PAD = KT3 * 128             # w1sel zero-padded so all tiles are full
    NB = MH + KT3 + 1              # packed bias columns

    nc = bacc.Bacc("TRN2", target_bir_lowering=False, num_devices=N_CORES)

    xT_d = nc.declare_dram_parameter("xT", [D * BL], CDT, isOutput=False)
    mask_d = nc.declare_dram_parameter("mask", [33, BL], _FP32, isOutput=False)
    wsh_d = nc.declare_dram_parameter("wsh", [D, H], CDT, isOutput=False)
    w1c_d = nc.declare_dram_parameter("w1c", [H, EF_PAD], CDT, isOutput=False)
    w2bd_d = nc.declare_dram_parameter("w2bd", [128, KT3 * ec], CDT, isOutput=False)
    bias_d = nc.declare_dram_parameter("biases", [128, NB], _FP32, isOutput=False)
    out_d = nc.declare_dram_parameter("out", [BL], _FP32, isOutput=True)

    relu = mybir.ActivationFunctionType.Relu
    sizes = CHUNK_SIZES
    offs = np.cumsum([0] + sizes).tolist()

    with TileContext(nc) as tc:
        with (
            tc.tile_pool(name="weights", bufs=1) as wpool,
            tc.tile_pool(name="xin", bufs=3) as xpool,
            tc.tile_pool(name="mid", bufs=3) as midpool,
            tc.tile_pool(name="small", bufs=3) as spool,
            tc.tile_pool(name="ps_h", bufs=4, space="PSUM") as ps_h,
            tc.tile_pool(name="ps_a", bufs=2, space="PSUM") as ps_a,
            tc.tile_pool(name="ps_c", bufs=1, space="PSUM") as ps_c,
            tc.tile_pool(name="ps_o", bufs=1, space="PSUM") as ps_o,
        ):
            _prio = [0]

            def pdma(q, dst, src):
                inst = q.dma_start(dst, src)
                inst.ins.bass_priority = _prio[0]
                _prio[0] += 1
                return inst

            def xview(c):
                sz = sizes[c]
                o = offs[c] * D
                return xT_d[o : o + D * sz].rearrange("(ko p t) -> p ko t", p=128, t=sz)

            wsh_view = wsh_d.rearrange("(o p) h -> p o h", p=128)
            wsh_ks = [wpool.tile([128, H], CDT, name=f"wshk{k}") for k in range(KD)]
            xt0_view = xview(0)
            xt0 = [
                xpool.tile([128, CHUNK], CDT, tag=f"xt{k}", name=f"xt0_{k}")
                for k in range(KD)
            ]
            for k in range(KD):
                qa = nc.sync if k % 2 == 0 else nc.scalar
                qb = nc.scalar if k % 2 == 0 else nc.sync
                pdma(qa, wsh_ks[k][:], wsh_view[:, k])
                pdma(qb, xt0[k][:, : sizes[0]], xt0_view[:, k])

            xts, masks = [[t[:, : sizes[0]] for t in xt0]], []
            w1c_ks = [None] * KH
            for c in range(len(sizes)):
                sz = sizes[c]
                if c > 0:
                    xv = xview(c)
                    xa = xpool.tile([128, KD // 2, CHUNK], CDT, tag="xta", name=f"xta{c}")
                    xb = xpool.tile([128, KD // 2, CHUNK], CDT, tag="xtb", name=f"xtb{c}")
                    pdma(nc.scalar, xa[:, :, :sz], xv[:, : KD // 2])
                    pdma(nc.sync, xb[:, :, :sz], xv[:, KD // 2 :])
                    xts.append([xa[:, k, :sz] for k in range(KD // 2)] + [xb[:, k, :sz] for k in range(KD // 2)])
                mask_sb = spool.tile([33, CHUNK], _FP32, tag="mask")
                pdma(nc.scalar, mask_sb[:, :sz], mask_d[:, offs[c] : offs[c] + sz])
                masks.append(mask_sb[:, :sz])
                if c == 0:
                    w1c_view = w1c_d.rearrange("(o p) f -> p o f", p=128)
                    for k in range(KH):
                        w1c_ks[k] = wpool.tile([128, EF_PAD], CDT, name=f"w1ck{k}")
                        pdma(nc.sync if k % 2 == 0 else nc.scalar, w1c_ks[k][:], w1c_view[:, k])
                    w2bd_sb = wpool.tile([128, KT3 * ec], CDT)
                    pdma(nc.sync, w2bd_sb[:], w2bd_d[:])
                    bias_sb = wpool.tile([128, NB], _FP32)
                    pdma(nc.sync, bias_sb[:], bias_d[:])
                    ones_sb = wpool.tile([ec, 1], CDT)
                    if COMPUTE_DT == "float32r":
                        nc.vector.memset(ones_sb[:].bitcast(mybir.dt.float32), 1.0)
                    else:
                        nc.vector.memset(ones_sb[:], 1.0)

            for c in range(len(sizes)):
                sz = sizes[c]
                t0 = offs[c]
                xt = xts[c]
                mask_sb = masks[c]

                hT = midpool.tile([128, MH, CHUNK], CDT, tag="hT", name=f"hT{c}")[:, :, :sz]
                if c == 0:
                    phs = [ps_h.tile([128, CHUNK], _FP32, tag="ps_h", name=f"ph{m}")[:, :sz] for m in range(MH)]
                    for k in range(KD):
                        for m in range(MH):
                            nc.tensor.matmul(
                                phs[m][:],
                                lhsT=wsh_ks[k][:, m * 128 : (m + 1) * 128],
                                rhs=xt[k][:],
                                start=(k == 0),
                                stop=(k == KD - 1),
                            )
                    for m in range(MH):
                        nc.scalar.activation(
                            hT[:, m, :], phs[m][:], relu, bias=bias_sb[:, m : m + 1]
                        )
                else:
                    for m in range(MH):
                        ph = ps_h.tile([128, CHUNK], _FP32, tag="ps_h", name=f"phx{c}_{m}")[:, :sz]
                        for k in range(KD):
                            nc.tensor.matmul(
                                ph[:],
                                lhsT=wsh_ks[k][:, m * 128 : (m + 1) * 128],
                                rhs=xt[k][:],
                                start=(k == 0),
                                stop=(k == KD - 1),
                            )
                        nc.scalar.activation(
                            hT[:, m, :], ph[:], relu, bias=bias_sb[:, m : m + 1]
                        )

                aT = midpool.tile([128, KT3, CHUNK], CDT, tag="aT", name=f"aT{c}")[:, :, :sz]
                for m in range(KT3):
                    f0 = m * 128
                    pa = ps_a.tile([128, CHUNK], _FP32, tag="ps_a", name=f"pa{c}_{m}")[:, :sz]
                    for k in range(KH):
                        nc.tensor.matmul(
                            pa[:],
                            lhsT=w1c_ks[k][:, f0 : f0 + 128],
                            rhs=hT[:, k, :],
                            start=(k == 0),
                            stop=(k == KH - 1),
                        )
                    nc.scalar.activation(
                        aT[:, m, :], pa[:], relu,
                        bias=bias_sb[:, MH + m : MH + m + 1],
                    )

                pc = ps_c.tile([ec, CHUNK], _FP32, tag="ps_c", name=f"pc{c}")[:, :sz]
                for k in range(KT3):
                    nc.tensor.matmul(
                        pc[:],
                        lhsT=w2bd_sb[:, k * ec : (k + 1) * ec],
                        rhs=aT[:, k, :],
                        start=(k == 0),
                        stop=(k == KT3 - 1),
                    )

                msel = spool.tile([ec, CHUNK], CDT, tag="msel", name=f"msel{c}")[:, :sz]
                nc.vector.tensor_mul(msel[:], pc[:], mask_sb[:ec])
                po = ps_o.tile([1, CHUNK], _FP32, tag="ps_o", name=f"po{c}")[:, :sz]
                nc.tensor.matmul(po[:], lhsT=ones_sb[:], rhs=msel[:], start=True, stop=True)
                ot = spool.tile([1, CHUNK], _FP32, tag="ot", name=f"ot{c}")[:, :sz]
                nc.vector.tensor_add(ot[:], po[:], mask_sb[32:33])
                nc.gpsimd.dma_start(out_d[t0 : t0 + sz].rearrange("(o t) -> o t", o=1), ot[:])

    nc.compile()
    return nc


def prepare_legacy(inputs):
    """Legacy host-side routing/sorting/sharding. Returns (ec, in_maps, unperm)."""
    np_dt = _np_in_dtype()
    x = np.asarray(inputs["x"], dtype=np.float32)
    idx = np.asarray(inputs["idx"]).astype(np.int64).reshape(B)
    W_shared = np.asarray(inputs["W_shared"], dtype=np.float32)
    b_shared = np.asarray(inputs["b_shared"], dtype=np.float32).reshape(H)
    W1 = np.asarray(inputs["W1"], dtype=np.float32)
    b1 = np.asarray(inputs["b1"], dtype=np.float32).reshape(E, F)
    W2 = np.asarray(inputs["W2"], dtype=np.float32).reshape(E, F)
    b2 = np.asarray(inputs["b2"], dtype=np.float32).reshape(E)
    send_to = np.asarray(inputs["send_to"]).astype(np.int64)

    perm = np.argsort(idx, kind="stable")
    idx_s = idx[perm]
    routes_s = send_to[idx_s]
    x_s = x[perm]

    expert_lists = []
    for c in range(N_CORES):
        sl = slice(c * BL, (c + 1) * BL)
        expert_lists.append(np.unique(routes_s[sl]))
    ec = max(EC_MIN, max(len(el) for el in expert_lists))
    ec = min(ec, E)

    wsh = np.ascontiguousarray(W_shared).astype(np_dt)
    EF = ec * F
    KT3 = (EF + 127) // 128
    EF_PAD = KT3 * 128
    NB = MH + KT3 + 1

    in_maps = []
    for c in range(N_CORES):
        sl = slice(c * BL, (c + 1) * BL)
        el = expert_lists[c]
        slots = np.full(ec, -1, dtype=np.int64)
        slots[: len(el)] = el

        r = routes_s[sl]
        mask = np.zeros((33, BL), dtype=np.float32)
        for k in range(r.shape[1]):
            hit = slots[:, None] == r[None, :, k]
            mask[:ec] += hit.astype(np.float32) / r.shape[1]
        mask[32] = b2[r].mean(axis=1)

        w1sel = np.zeros((H, EF_PAD), dtype=np.float32)
        b1sel = np.zeros(EF_PAD, dtype=np.float32)
        w2full = np.zeros((EF_PAD, ec), dtype=np.float32)
        for j, e in enumerate(slots):
            if e < 0:
                continue
            w1sel[:, j * F : (j + 1) * F] = W1[e]
            b1sel[j * F : (j + 1) * F] = b1[e]
            w2full[j * F : (j + 1) * F, j] = W2[e]
        w2bd = np.ascontiguousarray(
            w2full.reshape(KT3, 128, ec).transpose(1, 0, 2).reshape(128, KT3 * ec)
        ).astype(np_dt)

        biases = np.zeros((128, NB), dtype=np.float32)
        biases[:, :MH] = b_shared.reshape(MH, 128).T
        biases[:, MH : MH + KT3] = b1sel.reshape(KT3, 128).T
        biases[:ec, MH + KT3] = b2[np.maximum(slots, 0)] * (slots >= 0)

        xc = x_s[sl]
        parts, o = [], 0
        for szc in CHUNK_SIZES:
            parts.append(xc[o : o + szc].T.ravel())
            o += szc
        xT = np.ascontiguousarray(np.concatenate(parts)).astype(np_dt)

        in_maps.append(
            {
                "xT": xT,
                "mask": mask,
                "wsh": wsh,
                "w1c": w1sel.astype(np_dt),
                "w2bd": w2bd,
                "biases": biases,
            }
        )
    return ec, in_maps, perm


# ---------------------------------------------------------------------------
# public API
# ---------------------------------------------------------------------------


def prepare(inputs):
    """Returns (key, in_maps, unperm): actual[unperm] = concat(core outs)."""
    v2 = prepare_v2(inputs)
    if v2 is not None:
        in_maps, unperm = v2
        return ("v2",), in_maps, unperm
    ec, in_maps, perm = prepare_legacy(inputs)
    return ("legacy", ec), in_maps, perm


def get_nc(key):
    if key[0] == "v2":
        return get_nc_v2()
    ec = key[1]
    ckey = (COMPUTE_DT, "legacy", ec)
    if ckey not in _cache:
        _cache[ckey] = _build_nc_legacy(ec)
    return _cache[ckey]


def kernel(**inputs) -> np.ndarray:
    key, in_maps, unperm = prepare(inputs)
    nc = get_nc(key)
    res = run_bass_kernel_spmd(nc, in_maps, list(range(N_CORES)))
    out_sorted = np.concatenate([res.results[c]["out"] for c in range(N_CORES)])
    out = np.empty(B, dtype=np.float32)
    out[unperm] = out_sorted
    return out.reshape(B, 1)
